# revision 75
# baseline (speedup 1.0000x reference)
"""Trainium2 Bass kernel: transformer block with dilated (parity-strided,
banded, causal) attention, data-parallel over 8 cores (512 own tokens +
256 halo tokens per core).

v4 design (from v2 @ ~204us):
  * All GEMMs except scores/PV run fp8 e4m3 with perf_mode=DoubleRow
    (weights x64 on host, stationary operands packed block-major so every
    lhsT slice is per-partition contiguous).
  * LayerNorm is folded into the activation quantization itself:
    x_f8 = (x - mu) * rstd quantized on DVE (two tensor ops per tile pair)
    after a PE K=1 broadcast of the mu/rstd rows through the otherwise-idle
    ps_st PSUM ring.  GEMM epilogues collapse to a single ACT op
    (f(ps/64 + bias)) -- no DVE stt, no rank-1 chain tails, no negs terms.
  * LN stats are M=1 bf16 chains over x / x^2 (ACT Square), emitted
    chain-at-a-time in the DMA-wait shadow at kernel start.
  * All weights live in SBUF up front: K/Q block-major in 2 DMAs, V
    et-major in 1, out/ffn1/ffn2 in 2 gated DMAs (5.3MB) whose transfer is
    held back by a WAR hazard on a blocker tile consumed by the x_f8
    pipeline -- so they cannot contend with the critical x/qkv HBM loads
    (ungated they monopolized HBM for 15us, stalled the PE, and the HAM
    clock dropped to half speed for the whole kernel).
  * Attention (scores + additive -49152 mask matmul + exp + pipelined PV)
    is kept from v2; den extraction moved ACT->DVE; out-proj is fp8 DR
    with a partial chain overlapping late attention.
  * ACT table sets staged as in v2 (sqrt -> exp -> sqrt -> gelu) with
    dummy activations so the ~1.3us table loads stay off the critical path.
"""

import numpy as np
import ml_dtypes

import concourse.bass as bass
import concourse.bacc as bacc
import concourse.mybir as mybir
import concourse.tile as tile
from concourse.bass_utils import run_bass_kernel_spmd

BF16NP = ml_dtypes.bfloat16
F8NP = ml_dtypes.float8_e4m3
F32 = mybir.dt.float32
BF16 = mybir.dt.bfloat16
F8 = mybir.dt.float8e4
AF = mybir.ActivationFunctionType
OP = mybir.AluOpType
DR = mybir.MatmulPerfMode.DoubleRow

P = 128
B, L, E = 2, 2048, 768
ET = E // P            # 6 tiles over E
H, D = 12, 64
MLP = 4 * E            # 3072
MT = MLP // P          # 24
OWN = 512              # tokens owned per core
HALO = 256             # preceding-context tokens
SLAB = OWN + HALO      # 768
EPS = 1e-5
N_CORES = 8
MASKNEG = -49152.0     # exact in bf16; exp((s+MASKNEG)/8) == 0
CHUNKS1 = [(0, 512), (512, 256)]   # SLAB token chunks (PSUM bank = 512 f32)
CHUNKS2 = [(0, 512)]               # OWN token chunk

# wtail layout (bytes per partition into one [P, 41472] f8 tensor)
OW_OFF, OW_LEN = 0, ET * E              # 4608
W1_OFF, W1_LEN = 4608, ET * MLP         # 18432
W2_OFF, W2_LEN = 23040, MT * E          # 18432
WTAIL = 41472


def _fold2(apv):
    """[.., T] -> [.., 2, T//2] parity view of a stride-1 token axis."""
    return apv.rearrange("... (t two) -> ... two t", two=2)


def build_program():
    nc = bacc.Bacc("TRN2", target_bir_lowering=False, debug=False)

    xT = nc.dram_tensor("xT", [P, ET, SLAB], BF16, kind="ExternalInput").ap()
    wkqT = nc.dram_tensor("wkqT", [P, 12, ET, P], F8,
                          kind="ExternalInput").ap()
    wvT = nc.dram_tensor("wvT", [P, ET, E], F8, kind="ExternalInput").ap()
    wtailT = nc.dram_tensor("wtailT", [P, WTAIL], F8,
                            kind="ExternalInput").ap()
    cf32T = nc.dram_tensor("cf32T", [P, 182], F32, kind="ExternalInput").ap()
    cbf16T = nc.dram_tensor("cbf16T", [P, 1280], BF16,
                            kind="ExternalInput").ap()
    yT = nc.dram_tensor("yT", [P, ET, OWN], F32, kind="ExternalOutput").ap()

    with tile.TileContext(nc) as tc:
        _emit(tc, xT, wkqT, wvT, wtailT, cf32T, cbf16T, yT)
    nc.compile()
    return nc


def _emit(tc, xT, wkqT, wvT, wtailT, cf32T, cbf16T, yT):
    from contextlib import ExitStack
    from collections import deque
    ctx = ExitStack()
    nc = tc.nc

    sing = ctx.enter_context(tc.tile_pool(name="sing", bufs=1))
    ex_pool = ctx.enter_context(tc.tile_pool(name="ex", bufs=3))
    row_pool = ctx.enter_context(tc.tile_pool(name="rows", bufs=2))
    ft_pool = ctx.enter_context(tc.tile_pool(name="ftmp", bufs=3))
    den_pool = ctx.enter_context(tc.tile_pool(name="den", bufs=2))

    # PSUM: 8 banks total = g:3 + st:1 + sc:2 + pv:2
    ps_main = ctx.enter_context(tc.tile_pool(name="psg", bufs=3, space="PSUM"))
    ps_st = ctx.enter_context(tc.tile_pool(name="psst", bufs=1, space="PSUM"))
    ps_attn = ctx.enter_context(tc.tile_pool(name="pssc", bufs=2, space="PSUM"))
    ps_pv = ctx.enter_context(tc.tile_pool(name="pspv", bufs=2, space="PSUM"))

    # ---------------- phase 0: input DMAs + constants ----------------
    x_sb = sing.tile([P, ET, SLAB], BF16, tag="x_sb")
    for pr in range(3):
        nc.sync.dma_start(out=x_sb[:, 2 * pr:2 * pr + 2, :],
                          in_=xT[:, 2 * pr:2 * pr + 2, :])

    cf32_sb = sing.tile([P, 182], F32, tag="cf32")
    nc.sync.dma_start(out=cf32_sb, in_=cf32T)

    # K/Q weights block-major ([P, block, et, 128]) so every lhsT slice is
    # per-partition contiguous; V weights et-major (moving operand)
    wkq_sb = sing.tile([P, 12, ET, P], F8, tag="wkq")
    nc.sync.dma_start(out=wkq_sb[:, 0:6], in_=wkqT[:, 0:6])
    nc.sync.dma_start(out=wkq_sb[:, 6:12], in_=wkqT[:, 6:12])
    wv_sb = sing.tile([P, ET, E], F8, tag="wv")
    nc.sync.dma_start(out=wv_sb, in_=wvT)

    cbf16_sb = sing.tile([P, 1280], BF16, tag="cbf16")
    nc.sync.dma_start(out=cbf16_sb, in_=cbf16T)
    masks_sb = cbf16_sb[:, 0:1024].rearrange(
        "p (qb hr kb q) -> p qb hr kb q", qb=2, hr=2, kb=2)
    ident_sb = cbf16_sb[:, 1024:1152]
    ind2_sb = cbf16_sb[0:65, 1152:1280]
    # bias columns in cf32: K 0:6 | Q 6:12 | out 12:18 | b1 18:42 | b2 42:48

    ones_row = sing.tile([1, P], BF16, tag="ones_row")
    nc.vector.memset(ones_row, 1.0)
    ones_pf = sing.tile([P, 1], BF16, tag="ones_pf")
    nc.vector.memset(ones_pf, 1.0)
    eps_sb = sing.tile([1, 1], F32, tag="eps")
    nc.vector.memset(eps_sb, EPS)
    scr_sb = sing.tile([1, 4], F32, tag="scr")

    # preload the sqrt table set while input DMAs stream
    nc.scalar.activation(scr_sb[:, 0:1], eps_sb, AF.Sqrt)

    # dummy matmuls HAM-warm the PE clock while the input DMAs stream
    warm_src = sing.tile([P, 512], BF16, tag="warm_src")
    nc.gpsimd.memset(warm_src, 0.0)
    const_bf = nc.const_aps.aps[(mybir.dt.bfloat16, 1.0)]
    wps = ps_main.tile([P, 512], F32, tag="g", name="warm_ps")
    for wi in range(10):
        nc.tensor.matmul(wps[0:1, 0:512], const_bf, warm_src,
                         start=True, stop=True)

    def emit_warm_burst(n, name):
        # dependency-free matmuls straight into the PE queue: they execute
        # exactly when the FIFO reaches them, bridging dependency-wait gaps
        # so the HAM clock gate stays at 8/8 (uses the sc ring, which is
        # idle at both insertion points)
        wb = ps_attn.tile([P, 2, 2, P], F32, tag="sc", name=name)
        wv = wb.rearrange("p a b c -> p (a b c)")
        for wi in range(n):
            nc.tensor.matmul(wv[0:1, 0:512], const_bf, warm_src,
                             start=True, stop=True)

    # weight-tail prefetch, gated so its 5.3MB transfer cannot contend with
    # the critical x/qkv HBM loads: the xsq scratch shares the wtail's
    # 1-buf pool slot, so the WAR hazard holds the wtail DMAs until the
    # LN1 sq-chains have consumed x^2 (~15us in, exactly when HBM frees up)
    wt_pool = ctx.enter_context(tc.tile_pool(name="wt", bufs=1))
    xsq_host = wt_pool.tile([P, WTAIL // 2], BF16, tag="wt", name="xsq")
    xsq_sb = xsq_host[:, 0:ET * SLAB].rearrange("p (e t) -> p e t", e=ET)

    def emit_wtail_dma():
        wtail_sb = wt_pool.tile([P, WTAIL], F8, tag="wt", name="wtail")
        nc.sync.dma_start(out=wtail_sb[:, 0:W2_OFF], in_=wtailT[:, 0:W2_OFF])
        nc.sync.dma_start(out=wtail_sb[:, W2_OFF:], in_=wtailT[:, W2_OFF:])
        ow_v = wtail_sb[:, OW_OFF:OW_OFF + OW_LEN].rearrange(
            "p (e f c) -> p e f c", e=ET, f=ET)
        w1_v = wtail_sb[:, W1_OFF:W1_OFF + W1_LEN].rearrange(
            "p (m e c) -> p m e c", m=MT, e=ET)
        w2_v = wtail_sb[:, W2_OFF:W2_OFF + W2_LEN].rearrange(
            "p (e m c) -> p e m c", e=ET, m=MT)
        return ow_v, w1_v, w2_v

    # ---------------- phase 1: LN1 stats (bf16 M=1 chains) ----------------
    # squares on DVE (idle at t=0) so ACT reaches the mu copies -- and the
    # PE the mu broadcast -- the moment the stats x-chains land
    for pr in range(3):
        pa = slice(2 * pr, 2 * pr + 2)
        nc.vector.tensor_mul(xsq_sb[:, pa, :], x_sb[:, pa, :],
                             x_sb[:, pa, :])

    def emit_ln_stats(src, srcsq, chunks, name):
        st = ps_st.tile([P, 512], F32, tag="st", name=name)
        for data, roff in ((src, 0), (srcsq, 32)):
            for ci, (c0, cl) in enumerate(chunks):
                r0 = 64 * ci + roff
                for et in range(ET):
                    nc.tensor.matmul(
                        st[r0:r0 + 1, :cl], ones_pf,
                        data[:, et, c0:c0 + cl],
                        start=(et == 0), stop=(et == ET - 1),
                        tile_position=(0, r0))
        return st

    def emit_ln_mu(st, chunks, ninv, mu_row):
        for ci, (c0, cl) in enumerate(chunks):
            r0 = 64 * ci
            nc.scalar.activation(mu_row[:, c0:c0 + cl], st[r0:r0 + 1, :cl],
                                 AF.Copy, scale=ninv)

    def emit_ln_rstd(st, chunks, ninv, a_row):
        for ci, (c0, cl) in enumerate(chunks):
            r0 = 64 * ci
            musq = row_pool.tile([1, 512], F32, tag="row")
            nc.scalar.activation(musq[:, :cl], st[r0:r0 + 1, :cl], AF.Square,
                                 scale=ninv)
            var = row_pool.tile([1, 512], F32, tag="row")
            nc.vector.scalar_tensor_tensor(
                out=var[:, :cl], in0=st[r0 + 32:r0 + 33, :cl], scalar=ninv,
                in1=musq[:, :cl], op0=OP.mult, op1=OP.subtract)
            sd = row_pool.tile([1, 512], F32, tag="row")
            nc.scalar.activation(sd[:, :cl], var[:, :cl], AF.Sqrt,
                                 bias=eps_sb)
            af = row_pool.tile([1, 512], F32, tag="row")
            nc.vector.reciprocal_approx_fast(out=af[:, :cl], in_=sd[:, :cl])
            nc.vector.tensor_copy(out=a_row[:, c0:c0 + cl], in_=af[:, :cl])

    def emit_bc(rows_reps, chunks, name):
        # row -> all partitions via PE K=1 matmuls riding the idle sc ring
        # (the st ring would force the whole LN epilogue to finish first);
        # the PSUM->SBUF copies go on ACT so the DVE can start the
        # dependent normalize-quantize ops with no queued work ahead
        for ci, (c0, cl) in enumerate(chunks):
            for ri, (a_row, a_rep) in enumerate(rows_reps):
                bc = ps_attn.tile([P, 2, 2, P], F32, tag="sc",
                                  name=f"{name}{ci}{ri}")
                bcv = bc.rearrange("p a b c -> p (a b c)")
                nc.tensor.matmul(bcv[:, :cl], ones_row, a_row[:, c0:c0 + cl],
                                 start=True, stop=True)
                nc.scalar.copy(out=a_rep[:, c0:c0 + cl], in_=bcv[:, :cl])

    mu1 = sing.tile([1, SLAB], BF16, tag="mu1")
    a1row = sing.tile([1, SLAB], BF16, tag="a1row")
    mu1rep = sing.tile([P, SLAB], BF16, tag="mu1rep")
    a1rep = sing.tile([P, SLAB], BF16, tag="a1rep")
    st1 = emit_ln_stats(x_sb, xsq_sb, CHUNKS1, "st1")
    # mu broadcast fires as soon as the mu ACT lands; the centering subs
    # then overlap the rstd (sqrt/recip) path and the rstd broadcast
    emit_ln_mu(st1, CHUNKS1, 1.0 / E, mu1)
    emit_bc([(mu1, mu1rep)], CHUNKS1, "bc1m")
    # centering subs overlap the rstd (sqrt/recip) path on ACT; the sub/mul
    # emission interleave keeps the 4-buf xc ring deadlock-free (a sub may
    # only wait on a mul that is ahead of it in the DVE queue)
    x_f8 = sing.tile([P, ET, SLAB], F8, tag="x_f8")
    xcs = []
    for et in range(3):
        xc = ft_pool.tile([P, SLAB], BF16, tag="xc", bufs=4)
        nc.vector.tensor_sub(xc, x_sb[:, et, :], mu1rep)
        xcs.append(xc)
    emit_ln_rstd(st1, CHUNKS1, 1.0 / E, a1row)
    # prefetch the exp table set once LN1's rsqrt is done (before attention)
    nc.scalar.activation(scr_sb[:, 1:2], a1row[:, 0:1], AF.Exp)
    emit_warm_burst(6, "warmP")
    emit_bc([(a1row, a1rep)], CHUNKS1, "bc1a")
    for et in range(3, ET):
        nc.vector.tensor_mul(x_f8[:, et - 3, :], xcs[et - 3], a1rep)
        xc = ft_pool.tile([P, SLAB], BF16, tag="xc", bufs=4)
        nc.vector.tensor_sub(xc, x_sb[:, et, :], mu1rep)
        xcs.append(xc)
    for et in range(3, ET):
        nc.vector.tensor_mul(x_f8[:, et, :], xcs[et], a1rep)
    ow_v, w1_v, w2_v = emit_wtail_dma()
    emit_warm_burst(10, "warmK")

    # ---------------- phase 2: QKV projections (fp8 DR, LN pre-folded) ----
    k_sb = sing.tile([P, ET, SLAB], BF16, tag="k_sb")
    q_sb = sing.tile([P, ET, OWN], BF16, tag="q_sb")
    v_sb = sing.tile([P, 2, 3, H, D + 1], BF16, tag="v_sb")
    nc.vector.memset(v_sb[:, :, :, :, D:D + 1], 1.0)
    o_sb = sing.tile([P, ET, OWN], BF16, tag="o_sb")
    o_f8 = sing.tile([P, ET, OWN], F8, tag="o_f8")
    r_all = sing.tile([65, ET, 2, 256], BF16, tag="r_all")
    # rows 1-63 are never written but ARE read by the fused K=65 rrep
    # broadcast below (against ind2's zero rows); zero them once so
    # uninitialized SBUF can't contribute 0*NaN
    nc.gpsimd.memset(r_all, 0.0)
    raw_den = [sing.tile([1, ET, 2, 256], F32, tag=f"rawden{s}",
                         name=f"raw_den{s}") for s in range(2)]
    y1_sb = sing.tile([P, ET, OWN], F32, tag="y1_sb")
    y1b_sb = sing.tile([P, ET, OWN], BF16, tag="y1b")
    y1sq_sb = sing.tile([P, ET, OWN], BF16, tag="y1sq")
    y1_f8 = sing.tile([P, ET, OWN], F8, tag="y1_f8")
    ffnh = sing.tile([P, MT, OWN], F8, tag="ffnh")

    def emit_k(ft):
        for c0, cl in CHUNKS1:
            ps = ps_main.tile([P, 512], F32, tag="g", name=f"kps{ft}_{c0}")
            for e2 in range(0, ET, 2):
                nc.tensor.matmul(ps[:, :cl], wkq_sb[:, ft, e2:e2 + 2, :],
                                 x_f8[:, e2:e2 + 2, c0:c0 + cl],
                                 start=(e2 == 0), stop=(e2 == ET - 2),
                                 perf_mode=DR)
            # epilogue on DVE (not ACT) so attention-phase exps never queue
            nc.vector.tensor_scalar(
                out=k_sb[:, ft, c0:c0 + cl], in0=ps[:, :cl],
                scalar1=1.0 / 64.0, scalar2=cf32_sb[:, ft:ft + 1],
                op0=OP.mult, op1=OP.add)

    def emit_q(ft):
        ps = ps_main.tile([P, 512], F32, tag="g", name=f"qps{ft}")
        for e2 in range(0, ET, 2):
            nc.tensor.matmul(ps, wkq_sb[:, 6 + ft, e2:e2 + 2, :],
                             x_f8[:, e2:e2 + 2, HALO:SLAB],
                             start=(e2 == 0), stop=(e2 == ET - 2),
                             perf_mode=DR)
        nc.vector.tensor_scalar(
            out=q_sb[:, ft, :], in0=ps, scalar1=1.0 / 64.0,
            scalar2=cf32_sb[:, 6 + ft:7 + ft], op0=OP.mult, op1=OP.add)

    def emit_v(ci):
        # V in [token, feature] orientation; 1/64 weight descale via ACT
        vc0, vcl = [(0, 512), (512, 256)][ci]
        for kb in range(3):
            for par in range(2):
                ps = ps_main.tile([P, 512], F32, tag="g", name=f"vps{ci}")
                for e2 in range(0, ET, 2):
                    xblk = _fold2(x_f8[:, e2:e2 + 2, :])[:, :, par,
                                                         kb * P:(kb + 1) * P]
                    nc.tensor.matmul(
                        ps[:, :vcl], xblk,
                        wv_sb[:, e2:e2 + 2, vc0:vc0 + vcl],
                        start=(e2 == 0), stop=(e2 == ET - 2), perf_mode=DR)
                # V epilogue on DVE: the ACT queue must stay clear for the
                # attention exps (a queued V copy delays PSUM frees and
                # stalls the next chain's ring slot)
                nc.vector.tensor_scalar_mul(
                    v_sb[:, par, kb, vc0 // D:(vc0 + vcl) // D, 0:D],
                    ps[:, :vcl].rearrange("p (h d) -> p h d", d=D),
                    1.0 / 64.0)

    pairs = [(0, 2), (1, 3), (4, 6), (5, 7), (8, 10), (9, 11)]

    def emit_scores(pi, par, qb):
        h0, h1 = pairs[pi]
        ro = D * (h0 % 2)
        sc = ps_attn.tile([P, 2, 2, P], F32, tag="sc", name=f"sc{pi}{par}{qb}")
        nc.tensor.matmul(sc, ident_sb, masks_sb[:, qb],
                         start=True, stop=False)
        for hi, h in enumerate((h0, h1)):
            ktt = h // 2
            qv = _fold2(q_sb[ro:ro + D, ktt, :])[:, par, qb * P:(qb + 1) * P]
            kv = _fold2(k_sb[ro:ro + D, ktt, :])
            for kbi, kb in enumerate((qb, qb + 1)):
                nc.tensor.matmul(sc[:, hi, kbi, :],
                                 kv[:, par, kb * P:(kb + 1) * P], qv,
                                 start=False, stop=(hi == 1 and kbi == 1))
        ex = ex_pool.tile([P, 2, 2, P], BF16, tag="ex", name=f"ex{pi}{par}{qb}")
        nc.scalar.activation(ex, sc, AF.Exp, scale=1.0 / np.sqrt(D))
        return ex

    def emit_pv(pi, par, qb, ex):
        h0, h1 = pairs[pi]
        kt = h0 // 2
        ro = D * (h0 % 2)
        slot = h0 % 2
        pv = ps_pv.tile([D + 1, 2, P], F32, tag="pv")
        for hi, h in enumerate((h0, h1)):
            for kbi, kb in enumerate((qb, qb + 1)):
                nc.tensor.matmul(pv[:, hi, :], v_sb[:, par, kb, h, :],
                                 ex[:, hi, kbi, :],
                                 start=(hi == 0 and kbi == 0),
                                 stop=(hi == 1 and kbi == 1))
        nc.vector.tensor_copy(
            out=raw_den[slot][0:1, kt:kt + 2, par, qb * P:(qb + 1) * P],
            in_=pv[D:D + 1, :, :])
        dst = _fold2(o_sb[ro:ro + D, kt:kt + 2, :])[:, :, par,
                                                    qb * P:(qb + 1) * P]
        nc.scalar.copy(out=dst, in_=pv[0:D])

    def emit_attn_group(g):
        # software-pipelined: each block's PV trails two blocks behind its
        # scores so the strict-FIFO PE queue never stalls on an in-flight exp
        blocks = [(2 * g + s, par, qb) for s in (0, 1)
                  for par in range(2) for qb in range(2)]
        pending = deque()
        ex = None
        for blk in blocks:
            ex = emit_scores(*blk)
            pending.append((blk, ex))
            if len(pending) > 2:
                b, e = pending.popleft()
                emit_pv(*b, e)
        while pending:
            b, e = pending.popleft()
            emit_pv(*b, e)
        pi = 2 * g + 1
        kt = pairs[pi][0] // 2
        # batched denominator reciprocals: one recip per head-slot over the
        # whole group's [1, 1024] raw-den slice instead of 8 per-block ones
        for s in range(2):
            rc = den_pool.tile([1, 2, 2, 256], F32, tag="rcp", bufs=1)
            nc.vector.reciprocal_approx_fast(
                out=rc.rearrange("o a b c -> o (a b c)"),
                in_=raw_den[s][0:1, kt:kt + 2, :, :]
                .rearrange("o a b c -> o (a b c)"))
            nc.vector.tensor_copy(
                out=r_all[64 * s:64 * s + 1, kt:kt + 2, :, :], in_=rc)
        for tt in (kt, kt + 1):
            rrep = ps_main.tile([P, 512], F32, tag="g", name="rrep_ps")
            rrv = rrep.rearrange("m (a q) -> m a q", a=2)
            # single K=65 matmul: ind2 row 0 routes the slot-0 denominators
            # to partitions 0-63, row 64 routes slot-1 to 64-127; the zero
            # rows in between null out the unwritten r_all lanes
            nc.tensor.matmul(rrv, ind2_sb, r_all[0:65, tt],
                             start=True, stop=True)
            ofv = _fold2(o_sb[:, tt, :])
            of8v = _fold2(o_f8[:, tt, :])
            nc.vector.tensor_mul(of8v, ofv, rrv)
        if pi == len(pairs) - 1:
            # re-prefetch the sqrt set for LN2 after the final exp
            nc.scalar.activation(scr_sb[:, 2:3], ex[0:1, 0, 0, 0:1], AF.Sqrt)
        if pi == 3:
            # out-proj partial A over o feature tiles 0-3 (ready now), fp8 DR
            for et in range(ET):
                ps = ps_main.tile([P, 512], F32, tag="g", name="opA")
                for f2 in range(0, 4, 2):
                    nc.tensor.matmul(ps, ow_v[:, et, f2:f2 + 2, :],
                                     o_f8[:, f2:f2 + 2, :],
                                     start=(f2 == 0), stop=(f2 == 2),
                                     perf_mode=DR)
                t = ft_pool.tile([P, 512], F32, tag="ft")
                nc.scalar.activation(t, ps, AF.Identity, scale=1.0 / 64.0,
                                     bias=cf32_sb[:, 12 + et:13 + et])
                # residual add on gpsimd (idle during attention); frees the
                # DVE for the den/recip stream so opA's PSUM ring never gates
                nc.gpsimd.tensor_add(y1_sb[:, et, :], t,
                                     x_sb[:, et, HALO:SLAB])

    emit_k(0)
    emit_k(1)
    emit_warm_burst(4, "warmK2")
    emit_q(0)
    emit_q(1)
    emit_v(0)
    emit_warm_burst(8, "warmA")
    emit_attn_group(0)
    emit_k(2); emit_k(3)
    emit_q(2); emit_q(3)
    emit_attn_group(1)
    emit_k(4); emit_k(5)
    emit_q(4); emit_q(5)
    emit_v(1)
    emit_attn_group(2)

    # ------- phase 4: out-proj partial B + residual + inline LN2 stats ----
    # the stat-chain matmuls ride along per-et so the PE has real work
    # during the DVE/ACT-heavy opB epilogues and mu2 lands ~4us earlier
    st2 = ps_st.tile([P, 512], F32, tag="st", name="st2")
    for et in range(ET):
        ps = ps_main.tile([P, 512], F32, tag="g")
        nc.tensor.matmul(ps, ow_v[:, et, 4:6, :],
                         o_f8[:, 4:6, :], start=True, stop=True,
                         perf_mode=DR)
        nc.vector.scalar_tensor_tensor(
            out=y1_sb[:, et, :], in0=ps, scalar=1.0 / 64.0,
            in1=y1_sb[:, et, :], op0=OP.mult, op1=OP.add)
        nc.scalar.copy(out=y1b_sb[:, et, :], in_=y1_sb[:, et, :])
        nc.scalar.activation(y1sq_sb[:, et, :], y1b_sb[:, et, :], AF.Square)
        nc.tensor.matmul(st2[0:1, :], ones_pf, y1b_sb[:, et, :],
                         start=(et == 0), stop=(et == ET - 1),
                         tile_position=(0, 0))
        nc.tensor.matmul(st2[32:33, :], ones_pf, y1sq_sb[:, et, :],
                         start=(et == 0), stop=(et == ET - 1),
                         tile_position=(0, 32))
        if et in (1, 4):
            emit_warm_burst(3, "warmO")

    # ---------------- phase 5: LN2 epilogue ----------------
    mu2 = sing.tile([1, OWN], BF16, tag="mu2")
    a2row = sing.tile([1, OWN], BF16, tag="a2row")
    mu2rep = sing.tile([P, OWN], BF16, tag="mu2rep")
    a2rep = sing.tile([P, OWN], BF16, tag="a2rep")
    emit_ln_mu(st2, CHUNKS2, 1.0 / E, mu2)
    emit_bc([(mu2, mu2rep)], CHUNKS2, "bc2m")
    ycs = []
    for et in range(3):
        yc = ft_pool.tile([P, SLAB], BF16, tag="xc", bufs=4)
        nc.vector.tensor_sub(yc[:, :OWN], y1b_sb[:, et, :], mu2rep)
        ycs.append(yc)
    emit_ln_rstd(st2, CHUNKS2, 1.0 / E, a2row)
    # prefetch the gelu table set now that the last exp (rstd2) is emitted
    nc.scalar.activation(scr_sb[:, 3:4], a2row[:, 0:1], AF.Gelu)
    emit_warm_burst(4, "warmB")
    emit_bc([(a2row, a2rep)], CHUNKS2, "bc2a")
    for et in range(3, ET):
        nc.vector.tensor_mul(y1_f8[:, et - 3, :], ycs[et - 3][:, :OWN], a2rep)
        yc = ft_pool.tile([P, SLAB], BF16, tag="xc", bufs=4)
        nc.vector.tensor_sub(yc[:, :OWN], y1b_sb[:, et, :], mu2rep)
        ycs.append(yc)
    for et in range(3, ET):
        nc.vector.tensor_mul(y1_f8[:, et, :], ycs[et][:, :OWN], a2rep)
        if et == 4:
            emit_warm_burst(3, "warmY")

    # ---------------- phase 6: FFN1 + GELU (fp8 DR, LN pre-folded) --------
    for mt in range(MT):
        ps = ps_main.tile([P, 512], F32, tag="g", name=f"f1_{mt}")
        for e2 in range(0, ET, 2):
            nc.tensor.matmul(ps, w1_v[:, mt, e2:e2 + 2, :],
                             y1_f8[:, e2:e2 + 2, :],
                             start=(e2 == 0), stop=(e2 == ET - 2),
                             perf_mode=DR)
        nc.scalar.activation(ffnh[:, mt, :], ps, AF.Gelu, scale=1.0 / 64.0,
                             bias=cf32_sb[:, 18 + mt:19 + mt])

    # ---------------- phase 7: FFN2 + residual + store ----------------
    for et in range(ET):
        ps = ps_main.tile([P, 512], F32, tag="g")
        for k2 in range(0, MT, 2):
            nc.tensor.matmul(ps, w2_v[:, et, k2:k2 + 2, :],
                             ffnh[:, k2:k2 + 2, :],
                             start=(k2 == 0), stop=(k2 == MT - 2),
                             perf_mode=DR)
        t = ft_pool.tile([P, 512], F32, tag="ft")
        nc.scalar.activation(t, ps, AF.Identity, scale=1.0 / 64.0,
                             bias=cf32_sb[:, 42 + et:43 + et])
        nc.vector.tensor_add(y1_sb[:, et, :], t, y1_sb[:, et, :])
        nc.sync.dma_start(out=yT[:, et, :], in_=y1_sb[:, et, :])

    ctx.close()


# ======================= host side =======================

def _to_f8(w):
    return np.clip(w * 64.0, -240.0, 240.0).astype(F8NP)


def _pack_e(wT):
    """[E, C] (contraction-major) -> [P, ET, C] partition pack."""
    C = wT.shape[1]
    return np.ascontiguousarray(
        wT.reshape(ET, P, C).transpose(1, 0, 2))


def prep_inputs(x, ln1_w, ln1_b, qkv_w, qkv_b, out_w, out_b,
                ln2_w, ln2_b, ffn_w1, ffn_b1, ffn_w2, ffn_b2):
    """Shard/fold/cast the full inputs into 8 per-core input maps."""
    x = np.asarray(x, np.float32)
    f8 = lambda v: np.asarray(v, np.float64)

    def _blk(wf8, nblk):
        # [Ein, C] -> [P, C/128 blocks, Ein/128 tiles, 128] block-major pack
        ein, c = wf8.shape
        return np.ascontiguousarray(
            wf8.reshape(ein // P, P, nblk, c // nblk).transpose(1, 2, 0, 3))

    # qkv weights: fold ln1_w, transpose to [e, col], reorder cols K|Q|V
    qkv_wp = f8(qkv_w) * f8(ln1_w)[None, :]
    wT = qkv_wp.T                                   # [E, 3E], cols Q|K|V
    wT_r = np.concatenate([wT[:, E:2 * E], wT[:, 0:E], wT[:, 2 * E:]], axis=1)
    wqkv_f8 = _to_f8(wT_r)                          # [E, 3E] K|Q|V
    wkqT = _blk(wqkv_f8[:, 0:2 * E], 12)            # [P, 12, ET, 128]
    wvT = _pack_e(wqkv_f8[:, 2 * E:])               # [P, ET, E]

    ow_f8 = _to_f8(f8(out_w).T)                     # [E, E]
    owT = _blk(ow_f8, ET).reshape(P, OW_LEN)        # [P, et, fblk, 128]

    ffn_w1p = f8(ffn_w1) * f8(ln2_w)[None, :]
    w1_f8 = _to_f8(ffn_w1p.T)                       # [E, MLP]
    w1T = _blk(w1_f8, MT).reshape(P, W1_LEN)        # [P, mt, et, 128]

    w2_f8 = _to_f8(f8(ffn_w2).T)                    # [MLP, E]
    w2T = _blk(w2_f8, ET).reshape(P, W2_LEN)        # [P, et, mt, 128]

    wtail = np.concatenate([owT, w1T, w2T], axis=1)
    assert wtail.shape == (P, WTAIL)

    # biases (LN beta folded): K 0:6 | Q 6:12 | out 12:18 | b1 18:42 | b2
    qkv_b_eff = (f8(qkv_b) + f8(qkv_w) @ f8(ln1_b))
    out_b_eff = (f8(out_b) + f8(out_w) @ f8(qkv_b)[2 * E:])
    b1_eff = (f8(ffn_b1) + f8(ffn_w1) @ f8(ln2_b))
    cf32 = np.zeros((P, 182), np.float32)
    cf32[:, 0:6] = qkv_b_eff[E:2 * E].reshape(6, P).T
    cf32[:, 6:12] = qkv_b_eff[0:E].reshape(6, P).T
    cf32[:, 12:18] = out_b_eff.reshape(6, P).T
    cf32[:, 18:42] = b1_eff.reshape(MT, P).T
    cf32[:, 42:48] = np.asarray(ffn_b2, np.float32).reshape(6, P).T
    ind2 = np.zeros((65, P), np.float32)
    ind2[0, 0:D] = 1.0
    ind2[64, D:P] = 1.0
    cf32[0:65, 54:182] = ind2

    cidx = np.arange(P)[:, None]   # key (folded, within block)
    ridx = np.arange(P)[None, :]   # query (folded, within block)
    m_prev = np.where(cidx >= ridx, 0.0, MASKNEG).astype(BF16NP)
    m_diag = np.where(cidx <= ridx, 0.0, MASKNEG).astype(BF16NP)
    m_none = np.full((P, P), MASKNEG, BF16NP)

    in_maps = []
    for c in range(N_CORES):
        b, ch = divmod(c, 4)
        lo = OWN * ch - HALO
        if ch == 0:
            slab = np.concatenate(
                [np.zeros((HALO, E), np.float32), x[b, 0:OWN]], axis=0)
        else:
            slab = x[b, lo:lo + SLAB]
        xTc = np.ascontiguousarray(
            slab.T.reshape(ET, P, SLAB).transpose(1, 0, 2)).astype(BF16NP)

        # masks [key, qb, hrep, kb, q] additive
        mask = np.stack([
            np.stack([m_none if ch == 0 else m_prev, m_diag]),  # qb = 0
            np.stack([m_prev, m_diag]),                         # qb = 1
        ]).astype(BF16NP)          # [qb, kb, key, q]
        maskc = mask.transpose(2, 0, 1, 3)          # [key, qb, kb, q]
        maskc = np.broadcast_to(maskc[:, :, None], (P, 2, 2, 2, P))
        ind2b = np.zeros((P, P), BF16NP)
        ind2b[0, 0:D] = 1.0
        ind2b[64, D:P] = 1.0
        cbf16 = np.concatenate(
            [np.ascontiguousarray(maskc).reshape(P, 1024),
             np.eye(P, dtype=BF16NP), ind2b], axis=1).astype(BF16NP)

        in_maps.append({
            "xT": xTc, "wkqT": wkqT, "wvT": wvT, "wtailT": wtail,
            "cf32T": cf32, "cbf16T": cbf16,
        })
    return in_maps


def gather_output(results):
    y = np.empty((B, L, E), np.float32)
    for c in range(N_CORES):
        b, ch = divmod(c, 4)
        yc = results[c]["yT"]          # [P, ET, OWN]
        y[b, OWN * ch:OWN * (ch + 1)] = (
            yc.transpose(2, 1, 0).reshape(OWN, E))
    return y


_NC_CACHE = None


def _get_program():
    global _NC_CACHE
    if _NC_CACHE is None:
        _NC_CACHE = build_program()
    return _NC_CACHE


def kernel(**inputs):
    nc = _get_program()
    in_maps = prep_inputs(**inputs)
    res = run_bass_kernel_spmd(nc, in_maps, core_ids=list(range(N_CORES)))
    return gather_output(res.results)



# revision 77
# speedup vs baseline: 1.2139x; 1.2139x over previous
"""Trainium2 Bass kernel: transformer block with dilated (parity-strided,
banded, causal) attention, data-parallel over 8 cores (512 own tokens +
256 halo tokens per core).

v4 design (from v2 @ ~204us):
  * All GEMMs except scores/PV run fp8 e4m3 with perf_mode=DoubleRow
    (weights x64 on host, stationary operands packed block-major so every
    lhsT slice is per-partition contiguous).
  * LayerNorm is folded into the activation quantization itself:
    x_f8 = (x - mu) * rstd quantized on DVE (two tensor ops per tile pair)
    after a PE K=1 broadcast of the mu/rstd rows through the otherwise-idle
    ps_st PSUM ring.  GEMM epilogues collapse to a single ACT op
    (f(ps/64 + bias)) -- no DVE stt, no rank-1 chain tails, no negs terms.
  * LN stats are M=1 bf16 chains over x / x^2 (ACT Square), emitted
    chain-at-a-time in the DMA-wait shadow at kernel start.
  * All weights live in SBUF up front: K/Q block-major in 2 DMAs, V
    et-major in 1, out/ffn1/ffn2 in 2 gated DMAs (5.3MB) whose transfer is
    held back by a WAR hazard on a blocker tile consumed by the x_f8
    pipeline -- so they cannot contend with the critical x/qkv HBM loads
    (ungated they monopolized HBM for 15us, stalled the PE, and the HAM
    clock dropped to half speed for the whole kernel).
  * Attention (scores + additive -49152 mask matmul + exp + pipelined PV)
    is kept from v2; den extraction moved ACT->DVE; out-proj is fp8 DR
    with a partial chain overlapping late attention.
  * ACT table sets staged as in v2 (sqrt -> exp -> sqrt -> gelu) with
    dummy activations so the ~1.3us table loads stay off the critical path.
"""

import numpy as np
import ml_dtypes

import concourse.bass as bass
import concourse.bacc as bacc
import concourse.mybir as mybir
import concourse.tile as tile
from concourse.bass_utils import run_bass_kernel_spmd

BF16NP = ml_dtypes.bfloat16
F8NP = ml_dtypes.float8_e4m3
F32 = mybir.dt.float32
BF16 = mybir.dt.bfloat16
F8 = mybir.dt.float8e4
AF = mybir.ActivationFunctionType
OP = mybir.AluOpType
DR = mybir.MatmulPerfMode.DoubleRow

P = 128
B, L, E = 2, 2048, 768
ET = E // P            # 6 tiles over E
H, D = 12, 64
MLP = 4 * E            # 3072
MT = MLP // P          # 24
OWN = 512              # tokens owned per core
HALO = 256             # preceding-context tokens
SLAB = OWN + HALO      # 768
EPS = 1e-5
N_CORES = 8
MASKNEG = -49152.0     # exact in bf16; exp((s+MASKNEG)/8) == 0
CHUNKS1 = [(0, 512), (512, 256)]   # SLAB token chunks (PSUM bank = 512 f32)
CHUNKS2 = [(0, 512)]               # OWN token chunk

# wtail layout (bytes per partition into one [P, 41472] f8 tensor)
OW_OFF, OW_LEN = 0, ET * E              # 4608
W1_OFF, W1_LEN = 4608, ET * MLP         # 18432
W2_OFF, W2_LEN = 23040, MT * E          # 18432
WTAIL = 41472


def _fold2(apv):
    """[.., T] -> [.., 2, T//2] parity view of a stride-1 token axis."""
    return apv.rearrange("... (t two) -> ... two t", two=2)


def build_program():
    nc = bacc.Bacc("TRN2", target_bir_lowering=False, debug=False)

    xT = nc.dram_tensor("xT", [P, ET, SLAB], BF16, kind="ExternalInput").ap()
    wkqT = nc.dram_tensor("wkqT", [P, 12, ET, P], F8,
                          kind="ExternalInput").ap()
    wvT = nc.dram_tensor("wvT", [P, ET, E], F8, kind="ExternalInput").ap()
    wtailT = nc.dram_tensor("wtailT", [P, WTAIL], F8,
                            kind="ExternalInput").ap()
    cf32T = nc.dram_tensor("cf32T", [P, 182], F32, kind="ExternalInput").ap()
    cbf16T = nc.dram_tensor("cbf16T", [P, 1280], BF16,
                            kind="ExternalInput").ap()
    yT = nc.dram_tensor("yT", [P, ET, OWN], F32, kind="ExternalOutput").ap()

    with tile.TileContext(nc) as tc:
        _emit(tc, xT, wkqT, wvT, wtailT, cf32T, cbf16T, yT)
    nc.compile()
    return nc


def _emit(tc, xT, wkqT, wvT, wtailT, cf32T, cbf16T, yT):
    from contextlib import ExitStack
    from collections import deque
    ctx = ExitStack()
    nc = tc.nc

    sing = ctx.enter_context(tc.tile_pool(name="sing", bufs=1))
    ex_pool = ctx.enter_context(tc.tile_pool(name="ex", bufs=3))
    row_pool = ctx.enter_context(tc.tile_pool(name="rows", bufs=2))
    ft_pool = ctx.enter_context(tc.tile_pool(name="ftmp", bufs=3))
    den_pool = ctx.enter_context(tc.tile_pool(name="den", bufs=2))

    # PSUM: 8 banks total = g:3 + st:1 + sc:2 + pv:2
    ps_main = ctx.enter_context(tc.tile_pool(name="psg", bufs=3, space="PSUM"))
    ps_st = ctx.enter_context(tc.tile_pool(name="psst", bufs=1, space="PSUM"))
    ps_attn = ctx.enter_context(tc.tile_pool(name="pssc", bufs=2, space="PSUM"))
    ps_pv = ctx.enter_context(tc.tile_pool(name="pspv", bufs=2, space="PSUM"))

    # ---------------- phase 0: input DMAs + constants ----------------
    x_sb = sing.tile([P, ET, SLAB], BF16, tag="x_sb")
    for pr in range(3):
        nc.sync.dma_start(out=x_sb[:, 2 * pr:2 * pr + 2, :],
                          in_=xT[:, 2 * pr:2 * pr + 2, :])

    cf32_sb = sing.tile([P, 182], F32, tag="cf32")
    nc.sync.dma_start(out=cf32_sb, in_=cf32T)

    # K/Q weights block-major ([P, block, et, 128]) so every lhsT slice is
    # per-partition contiguous; V weights et-major (moving operand)
    wkq_sb = sing.tile([P, 12, ET, P], F8, tag="wkq")
    nc.sync.dma_start(out=wkq_sb[:, 0:6], in_=wkqT[:, 0:6])
    nc.sync.dma_start(out=wkq_sb[:, 6:12], in_=wkqT[:, 6:12])
    wv_sb = sing.tile([P, ET, E], F8, tag="wv")
    nc.sync.dma_start(out=wv_sb, in_=wvT)

    cbf16_sb = sing.tile([P, 1280], BF16, tag="cbf16")
    nc.sync.dma_start(out=cbf16_sb, in_=cbf16T)
    masks_sb = cbf16_sb[:, 0:1024].rearrange(
        "p (qb hr kb q) -> p qb hr kb q", qb=2, hr=2, kb=2)
    ident_sb = cbf16_sb[:, 1024:1152]
    ind2_sb = cbf16_sb[0:65, 1152:1280]
    # bias columns in cf32: K 0:6 | Q 6:12 | out 12:18 | b1 18:42 | b2 42:48

    ones_row = sing.tile([1, P], BF16, tag="ones_row")
    nc.vector.memset(ones_row, 1.0)
    ones_pf = sing.tile([P, 1], BF16, tag="ones_pf")
    nc.vector.memset(ones_pf, 1.0)
    eps_sb = sing.tile([1, 1], F32, tag="eps")
    nc.vector.memset(eps_sb, EPS)
    scr_sb = sing.tile([1, 4], F32, tag="scr")

    # preload the sqrt table set while input DMAs stream
    nc.scalar.activation(scr_sb[:, 0:1], eps_sb, AF.Sqrt)

    # dummy matmuls HAM-warm the PE clock while the input DMAs stream
    warm_src = sing.tile([P, 512], BF16, tag="warm_src")
    nc.gpsimd.memset(warm_src, 0.0)
    const_bf = nc.const_aps.aps[(mybir.dt.bfloat16, 1.0)]
    wps = ps_main.tile([P, 512], F32, tag="g", name="warm_ps")
    for wi in range(10):
        nc.tensor.matmul(wps[0:1, 0:512], const_bf, warm_src,
                         start=True, stop=True)

    def emit_warm_burst(n, name):
        # dependency-free matmuls straight into the PE queue: they execute
        # exactly when the FIFO reaches them, bridging dependency-wait gaps
        # so the HAM clock gate stays at 8/8 (uses the sc ring, which is
        # idle at both insertion points)
        wb = ps_attn.tile([P, 2, 2, P], F32, tag="sc", name=name)
        wv = wb.rearrange("p a b c -> p (a b c)")
        for wi in range(n):
            nc.tensor.matmul(wv[0:1, 0:512], const_bf, warm_src,
                             start=True, stop=True)

    # weight-tail prefetch, gated so its 5.3MB transfer cannot contend with
    # the critical x/qkv HBM loads: the xsq scratch shares the wtail's
    # 1-buf pool slot, so the WAR hazard holds the wtail DMAs until the
    # LN1 sq-chains have consumed x^2 (~15us in, exactly when HBM frees up)
    wt_pool = ctx.enter_context(tc.tile_pool(name="wt", bufs=1))
    xsq_host = wt_pool.tile([P, WTAIL // 2], BF16, tag="wt", name="xsq")
    xsq_sb = xsq_host[:, 0:ET * SLAB].rearrange("p (e t) -> p e t", e=ET)

    def emit_wtail_dma():
        wtail_sb = wt_pool.tile([P, WTAIL], F8, tag="wt", name="wtail")
        nc.sync.dma_start(out=wtail_sb[:, 0:W2_OFF], in_=wtailT[:, 0:W2_OFF])
        nc.sync.dma_start(out=wtail_sb[:, W2_OFF:], in_=wtailT[:, W2_OFF:])
        ow_v = wtail_sb[:, OW_OFF:OW_OFF + OW_LEN].rearrange(
            "p (e f c) -> p e f c", e=ET, f=ET)
        w1_v = wtail_sb[:, W1_OFF:W1_OFF + W1_LEN].rearrange(
            "p (m e c) -> p m e c", m=MT, e=ET)
        w2_v = wtail_sb[:, W2_OFF:W2_OFF + W2_LEN].rearrange(
            "p (e m c) -> p e m c", e=ET, m=MT)
        return ow_v, w1_v, w2_v

    # ---------------- phase 1: LN1 stats (bf16 M=1 chains) ----------------
    # squares on DVE (idle at t=0) so ACT reaches the mu copies -- and the
    # PE the mu broadcast -- the moment the stats x-chains land
    for pr in range(3):
        pa = slice(2 * pr, 2 * pr + 2)
        nc.vector.tensor_mul(xsq_sb[:, pa, :], x_sb[:, pa, :],
                             x_sb[:, pa, :])

    def emit_ln_stats(src, srcsq, chunks, name):
        st = ps_st.tile([P, 512], F32, tag="st", name=name)
        for data, roff in ((src, 0), (srcsq, 32)):
            for ci, (c0, cl) in enumerate(chunks):
                r0 = 64 * ci + roff
                for et in range(ET):
                    nc.tensor.matmul(
                        st[r0:r0 + 1, :cl], ones_pf,
                        data[:, et, c0:c0 + cl],
                        start=(et == 0), stop=(et == ET - 1),
                        tile_position=(0, r0))
        return st

    def emit_ln_mu(st, chunks, ninv, mu_row):
        for ci, (c0, cl) in enumerate(chunks):
            r0 = 64 * ci
            nc.scalar.activation(mu_row[:, c0:c0 + cl], st[r0:r0 + 1, :cl],
                                 AF.Copy, scale=ninv)

    def emit_ln_rstd(st, chunks, ninv, a_row):
        for ci, (c0, cl) in enumerate(chunks):
            r0 = 64 * ci
            musq = row_pool.tile([1, 512], F32, tag="row")
            nc.scalar.activation(musq[:, :cl], st[r0:r0 + 1, :cl], AF.Square,
                                 scale=ninv)
            var = row_pool.tile([1, 512], F32, tag="row")
            nc.vector.scalar_tensor_tensor(
                out=var[:, :cl], in0=st[r0 + 32:r0 + 33, :cl], scalar=ninv,
                in1=musq[:, :cl], op0=OP.mult, op1=OP.subtract)
            sd = row_pool.tile([1, 512], F32, tag="row")
            nc.scalar.activation(sd[:, :cl], var[:, :cl], AF.Sqrt,
                                 bias=eps_sb)
            af = row_pool.tile([1, 512], F32, tag="row")
            nc.vector.reciprocal_approx_fast(out=af[:, :cl], in_=sd[:, :cl])
            nc.vector.tensor_copy(out=a_row[:, c0:c0 + cl], in_=af[:, :cl])

    def emit_bc(rows_reps, chunks, name):
        # row -> all partitions via PE K=1 matmuls riding the idle sc ring
        # (the st ring would force the whole LN epilogue to finish first);
        # the PSUM->SBUF copies go on ACT so the DVE can start the
        # dependent normalize-quantize ops with no queued work ahead
        for ci, (c0, cl) in enumerate(chunks):
            for ri, (a_row, a_rep) in enumerate(rows_reps):
                bc = ps_attn.tile([P, 2, 2, P], F32, tag="sc",
                                  name=f"{name}{ci}{ri}")
                bcv = bc.rearrange("p a b c -> p (a b c)")
                nc.tensor.matmul(bcv[:, :cl], ones_row, a_row[:, c0:c0 + cl],
                                 start=True, stop=True)
                nc.scalar.copy(out=a_rep[:, c0:c0 + cl], in_=bcv[:, :cl])

    mu1 = sing.tile([1, SLAB], BF16, tag="mu1")
    a1row = sing.tile([1, SLAB], BF16, tag="a1row")
    mu1rep = sing.tile([P, SLAB], BF16, tag="mu1rep")
    a1rep = sing.tile([P, SLAB], BF16, tag="a1rep")
    st1 = emit_ln_stats(x_sb, xsq_sb, CHUNKS1, "st1")
    # mu broadcast fires as soon as the mu ACT lands; the centering subs
    # then overlap the rstd (sqrt/recip) path and the rstd broadcast
    emit_ln_mu(st1, CHUNKS1, 1.0 / E, mu1)
    emit_bc([(mu1, mu1rep)], CHUNKS1, "bc1m")
    # centering subs overlap the rstd (sqrt/recip) path on ACT; the sub/mul
    # emission interleave keeps the 4-buf xc ring deadlock-free (a sub may
    # only wait on a mul that is ahead of it in the DVE queue)
    x_f8 = sing.tile([P, ET, SLAB], F8, tag="x_f8")
    xcs = []
    for et in range(3):
        xc = ft_pool.tile([P, SLAB], BF16, tag="xc", bufs=4)
        nc.vector.tensor_sub(xc, x_sb[:, et, :], mu1rep)
        xcs.append(xc)
    emit_ln_rstd(st1, CHUNKS1, 1.0 / E, a1row)
    # prefetch the exp table set once LN1's rsqrt is done (before attention)
    nc.scalar.activation(scr_sb[:, 1:2], a1row[:, 0:1], AF.Exp)
    emit_warm_burst(6, "warmP")
    emit_bc([(a1row, a1rep)], CHUNKS1, "bc1a")
    for et in range(3, ET):
        nc.vector.tensor_mul(x_f8[:, et - 3, :], xcs[et - 3], a1rep)
        xc = ft_pool.tile([P, SLAB], BF16, tag="xc", bufs=4)
        nc.vector.tensor_sub(xc, x_sb[:, et, :], mu1rep)
        xcs.append(xc)
    for et in range(3, ET):
        nc.vector.tensor_mul(x_f8[:, et, :], xcs[et], a1rep)
    ow_v, w1_v, w2_v = emit_wtail_dma()
    emit_warm_burst(10, "warmK")

    # ---------------- phase 2: QKV projections (fp8 DR, LN pre-folded) ----
    k_sb = sing.tile([P, ET, SLAB], BF16, tag="k_sb")
    q_sb = sing.tile([P, ET, OWN], BF16, tag="q_sb")
    v_sb = sing.tile([P, 2, 3, H, D + 1], BF16, tag="v_sb")
    nc.vector.memset(v_sb[:, :, :, :, D:D + 1], 1.0)
    o_sb = sing.tile([P, ET, OWN], BF16, tag="o_sb")
    o_f8 = sing.tile([P, ET, OWN], F8, tag="o_f8")
    r_all = sing.tile([65, ET, 2, 256], BF16, tag="r_all")
    # rows 1-63 are never written but ARE read by the fused K=65 rrep
    # broadcast below (against ind2's zero rows); zero them once so
    # uninitialized SBUF can't contribute 0*NaN
    nc.gpsimd.memset(r_all, 0.0)
    raw_den = [sing.tile([1, ET, 2, 256], F32, tag=f"rawden{s}",
                         name=f"raw_den{s}") for s in range(2)]
    y1_sb = sing.tile([P, ET, OWN], F32, tag="y1_sb")
    y1b_sb = sing.tile([P, ET, OWN], BF16, tag="y1b")
    y1sq_sb = sing.tile([P, ET, OWN], BF16, tag="y1sq")
    y1_f8 = sing.tile([P, ET, OWN], F8, tag="y1_f8")
    ffnh = sing.tile([P, MT, OWN], F8, tag="ffnh")

    def emit_k(ft):
        for c0, cl in CHUNKS1:
            ps = ps_main.tile([P, 512], F32, tag="g", name=f"kps{ft}_{c0}")
            for e2 in range(0, ET, 2):
                nc.tensor.matmul(ps[:, :cl], wkq_sb[:, ft, e2:e2 + 2, :],
                                 x_f8[:, e2:e2 + 2, c0:c0 + cl],
                                 start=(e2 == 0), stop=(e2 == ET - 2),
                                 perf_mode=DR)
            # epilogue on DVE (not ACT) so attention-phase exps never queue
            nc.vector.tensor_scalar(
                out=k_sb[:, ft, c0:c0 + cl], in0=ps[:, :cl],
                scalar1=1.0 / 64.0, scalar2=cf32_sb[:, ft:ft + 1],
                op0=OP.mult, op1=OP.add)

    def emit_q(ft):
        ps = ps_main.tile([P, 512], F32, tag="g", name=f"qps{ft}")
        for e2 in range(0, ET, 2):
            nc.tensor.matmul(ps, wkq_sb[:, 6 + ft, e2:e2 + 2, :],
                             x_f8[:, e2:e2 + 2, HALO:SLAB],
                             start=(e2 == 0), stop=(e2 == ET - 2),
                             perf_mode=DR)
        nc.vector.tensor_scalar(
            out=q_sb[:, ft, :], in0=ps, scalar1=1.0 / 64.0,
            scalar2=cf32_sb[:, 6 + ft:7 + ft], op0=OP.mult, op1=OP.add)

    def emit_v(ci):
        # V in [token, feature] orientation; 1/64 weight descale via ACT
        vc0, vcl = [(0, 512), (512, 256)][ci]
        for kb in range(3):
            for par in range(2):
                ps = ps_main.tile([P, 512], F32, tag="g", name=f"vps{ci}")
                for e2 in range(0, ET, 2):
                    xblk = _fold2(x_f8[:, e2:e2 + 2, :])[:, :, par,
                                                         kb * P:(kb + 1) * P]
                    nc.tensor.matmul(
                        ps[:, :vcl], xblk,
                        wv_sb[:, e2:e2 + 2, vc0:vc0 + vcl],
                        start=(e2 == 0), stop=(e2 == ET - 2), perf_mode=DR)
                # V epilogue on DVE: the ACT queue must stay clear for the
                # attention exps (a queued V copy delays PSUM frees and
                # stalls the next chain's ring slot)
                nc.vector.tensor_scalar_mul(
                    v_sb[:, par, kb, vc0 // D:(vc0 + vcl) // D, 0:D],
                    ps[:, :vcl].rearrange("p (h d) -> p h d", d=D),
                    1.0 / 64.0)

    pairs = [(0, 2), (1, 3), (4, 6), (5, 7), (8, 10), (9, 11)]

    def emit_scores(pi, par, qb):
        h0, h1 = pairs[pi]
        ro = D * (h0 % 2)
        sc = ps_attn.tile([P, 2, 2, P], F32, tag="sc", name=f"sc{pi}{par}{qb}")
        nc.tensor.matmul(sc, ident_sb, masks_sb[:, qb],
                         start=True, stop=False)
        for hi, h in enumerate((h0, h1)):
            ktt = h // 2
            qv = _fold2(q_sb[ro:ro + D, ktt, :])[:, par, qb * P:(qb + 1) * P]
            kv = _fold2(k_sb[ro:ro + D, ktt, :])
            for kbi, kb in enumerate((qb, qb + 1)):
                nc.tensor.matmul(sc[:, hi, kbi, :],
                                 kv[:, par, kb * P:(kb + 1) * P], qv,
                                 start=False, stop=(hi == 1 and kbi == 1))
        ex = ex_pool.tile([P, 2, 2, P], BF16, tag="ex", name=f"ex{pi}{par}{qb}")
        nc.scalar.activation(ex, sc, AF.Exp, scale=1.0 / np.sqrt(D))
        return ex

    def emit_pv(pi, par, qb, ex):
        h0, h1 = pairs[pi]
        kt = h0 // 2
        ro = D * (h0 % 2)
        slot = h0 % 2
        pv = ps_pv.tile([D + 1, 2, P], F32, tag="pv")
        for hi, h in enumerate((h0, h1)):
            for kbi, kb in enumerate((qb, qb + 1)):
                nc.tensor.matmul(pv[:, hi, :], v_sb[:, par, kb, h, :],
                                 ex[:, hi, kbi, :],
                                 start=(hi == 0 and kbi == 0),
                                 stop=(hi == 1 and kbi == 1))
        nc.vector.tensor_copy(
            out=raw_den[slot][0:1, kt:kt + 2, par, qb * P:(qb + 1) * P],
            in_=pv[D:D + 1, :, :])
        dst = _fold2(o_sb[ro:ro + D, kt:kt + 2, :])[:, :, par,
                                                    qb * P:(qb + 1) * P]
        nc.scalar.copy(out=dst, in_=pv[0:D])

    def emit_attn_blocks(g):
        # software-pipelined: each block's PV trails two blocks behind its
        # scores so the strict-FIFO PE queue never stalls on an in-flight exp
        blocks = [(2 * g + s, par, qb) for s in (0, 1)
                  for par in range(2) for qb in range(2)]
        pending = deque()
        ex = None
        for blk in blocks:
            ex = emit_scores(*blk)
            pending.append((blk, ex))
            if len(pending) > 2:
                b, e = pending.popleft()
                emit_pv(*b, e)
        while pending:
            b, e = pending.popleft()
            emit_pv(*b, e)
        if g == 2:
            # re-prefetch the sqrt set for LN2 after the final exp
            nc.scalar.activation(scr_sb[:, 2:3], ex[0:1, 0, 0, 0:1], AF.Sqrt)

    def emit_attn_tail(g):
        # emitted AFTER the next group's K chains: the recips/rreps wait on
        # the DVE den stream, and the interposed chains keep the PE fed
        pi = 2 * g + 1
        kt = pairs[pi][0] // 2
        # batched denominator reciprocals: one recip per head-slot over the
        # whole group's [1, 1024] raw-den slice instead of 8 per-block ones
        for s in range(2):
            rc = den_pool.tile([1, 2, 2, 256], F32, tag="rcp", bufs=1)
            nc.vector.reciprocal_approx_fast(
                out=rc.rearrange("o a b c -> o (a b c)"),
                in_=raw_den[s][0:1, kt:kt + 2, :, :]
                .rearrange("o a b c -> o (a b c)"))
            nc.vector.tensor_copy(
                out=r_all[64 * s:64 * s + 1, kt:kt + 2, :, :], in_=rc)
        for tt in (kt, kt + 1):
            rrep = ps_main.tile([P, 512], F32, tag="g", name="rrep_ps")
            rrv = rrep.rearrange("m (a q) -> m a q", a=2)
            # single K=65 matmul: ind2 row 0 routes the slot-0 denominators
            # to partitions 0-63, row 64 routes slot-1 to 64-127; the zero
            # rows in between null out the unwritten r_all lanes
            nc.tensor.matmul(rrv, ind2_sb, r_all[0:65, tt],
                             start=True, stop=True)
            ofv = _fold2(o_sb[:, tt, :])
            of8v = _fold2(o_f8[:, tt, :])
            nc.vector.tensor_mul(of8v, ofv, rrv)
        if pi == 3:
            # out-proj partial A over o feature tiles 0-3 (ready now), fp8 DR
            for et in range(ET):
                ps = ps_main.tile([P, 512], F32, tag="g", name="opA")
                for f2 in range(0, 4, 2):
                    nc.tensor.matmul(ps, ow_v[:, et, f2:f2 + 2, :],
                                     o_f8[:, f2:f2 + 2, :],
                                     start=(f2 == 0), stop=(f2 == 2),
                                     perf_mode=DR)
                t = ft_pool.tile([P, 512], F32, tag="ft")
                nc.scalar.activation(t, ps, AF.Identity, scale=1.0 / 64.0,
                                     bias=cf32_sb[:, 12 + et:13 + et])
                # residual add on gpsimd (idle during attention); frees the
                # DVE for the den/recip stream so opA's PSUM ring never gates
                nc.gpsimd.tensor_add(y1_sb[:, et, :], t,
                                     x_sb[:, et, HALO:SLAB])

    emit_k(0)
    emit_k(1)
    emit_warm_burst(4, "warmK2")
    emit_q(0)
    emit_q(1)
    emit_v(0)
    emit_warm_burst(8, "warmA")
    emit_attn_blocks(0)
    emit_k(2); emit_k(3)
    emit_attn_tail(0)
    emit_q(2); emit_q(3)
    emit_attn_blocks(1)
    emit_k(4); emit_k(5)
    emit_attn_tail(1)
    emit_q(4); emit_q(5)
    emit_v(1)
    emit_attn_blocks(2)
    emit_attn_tail(2)

    # ------- phase 4: out-proj partial B + residual + inline LN2 stats ----
    # the stat-chain matmuls ride along per-et so the PE has real work
    # during the DVE/ACT-heavy opB epilogues and mu2 lands ~4us earlier
    st2 = ps_st.tile([P, 512], F32, tag="st", name="st2")
    for et in range(ET):
        ps = ps_main.tile([P, 512], F32, tag="g")
        nc.tensor.matmul(ps, ow_v[:, et, 4:6, :],
                         o_f8[:, 4:6, :], start=True, stop=True,
                         perf_mode=DR)
        nc.vector.scalar_tensor_tensor(
            out=y1_sb[:, et, :], in0=ps, scalar=1.0 / 64.0,
            in1=y1_sb[:, et, :], op0=OP.mult, op1=OP.add)
        nc.scalar.copy(out=y1b_sb[:, et, :], in_=y1_sb[:, et, :])
        nc.scalar.activation(y1sq_sb[:, et, :], y1b_sb[:, et, :], AF.Square)
        nc.tensor.matmul(st2[0:1, :], ones_pf, y1b_sb[:, et, :],
                         start=(et == 0), stop=(et == ET - 1),
                         tile_position=(0, 0))
        nc.tensor.matmul(st2[32:33, :], ones_pf, y1sq_sb[:, et, :],
                         start=(et == 0), stop=(et == ET - 1),
                         tile_position=(0, 32))
        if et in (1, 4):
            emit_warm_burst(3, "warmO")

    # ---------------- phase 5: LN2 epilogue ----------------
    mu2 = sing.tile([1, OWN], BF16, tag="mu2")
    a2row = sing.tile([1, OWN], BF16, tag="a2row")
    mu2rep = sing.tile([P, OWN], BF16, tag="mu2rep")
    a2rep = sing.tile([P, OWN], BF16, tag="a2rep")
    emit_ln_mu(st2, CHUNKS2, 1.0 / E, mu2)
    emit_bc([(mu2, mu2rep)], CHUNKS2, "bc2m")
    ycs = []
    for et in range(3):
        yc = ft_pool.tile([P, SLAB], BF16, tag="xc", bufs=4)
        nc.vector.tensor_sub(yc[:, :OWN], y1b_sb[:, et, :], mu2rep)
        ycs.append(yc)
    emit_ln_rstd(st2, CHUNKS2, 1.0 / E, a2row)
    # prefetch the gelu table set now that the last exp (rstd2) is emitted
    nc.scalar.activation(scr_sb[:, 3:4], a2row[:, 0:1], AF.Gelu)
    emit_warm_burst(4, "warmB")
    emit_bc([(a2row, a2rep)], CHUNKS2, "bc2a")
    for et in range(3, ET):
        nc.vector.tensor_mul(y1_f8[:, et - 3, :], ycs[et - 3][:, :OWN], a2rep)
        yc = ft_pool.tile([P, SLAB], BF16, tag="xc", bufs=4)
        nc.vector.tensor_sub(yc[:, :OWN], y1b_sb[:, et, :], mu2rep)
        ycs.append(yc)
    for et in range(3, ET):
        nc.vector.tensor_mul(y1_f8[:, et, :], ycs[et][:, :OWN], a2rep)
        if et == 4:
            emit_warm_burst(3, "warmY")

    # ---------------- phase 6: FFN1 + GELU (fp8 DR, LN pre-folded) --------
    for mt in range(MT):
        ps = ps_main.tile([P, 512], F32, tag="g", name=f"f1_{mt}")
        for e2 in range(0, ET, 2):
            nc.tensor.matmul(ps, w1_v[:, mt, e2:e2 + 2, :],
                             y1_f8[:, e2:e2 + 2, :],
                             start=(e2 == 0), stop=(e2 == ET - 2),
                             perf_mode=DR)
        nc.scalar.activation(ffnh[:, mt, :], ps, AF.Gelu, scale=1.0 / 64.0,
                             bias=cf32_sb[:, 18 + mt:19 + mt])

    # ---------------- phase 7: FFN2 + residual + store ----------------
    for et in range(ET):
        ps = ps_main.tile([P, 512], F32, tag="g")
        for k2 in range(0, MT, 2):
            nc.tensor.matmul(ps, w2_v[:, et, k2:k2 + 2, :],
                             ffnh[:, k2:k2 + 2, :],
                             start=(k2 == 0), stop=(k2 == MT - 2),
                             perf_mode=DR)
        t = ft_pool.tile([P, 512], F32, tag="ft")
        nc.scalar.activation(t, ps, AF.Identity, scale=1.0 / 64.0,
                             bias=cf32_sb[:, 42 + et:43 + et])
        nc.vector.tensor_add(y1_sb[:, et, :], t, y1_sb[:, et, :])
        nc.sync.dma_start(out=yT[:, et, :], in_=y1_sb[:, et, :])

    ctx.close()


# ======================= host side =======================

def _to_f8(w):
    return np.clip(w * 64.0, -240.0, 240.0).astype(F8NP)


def _pack_e(wT):
    """[E, C] (contraction-major) -> [P, ET, C] partition pack."""
    C = wT.shape[1]
    return np.ascontiguousarray(
        wT.reshape(ET, P, C).transpose(1, 0, 2))


def prep_inputs(x, ln1_w, ln1_b, qkv_w, qkv_b, out_w, out_b,
                ln2_w, ln2_b, ffn_w1, ffn_b1, ffn_w2, ffn_b2):
    """Shard/fold/cast the full inputs into 8 per-core input maps."""
    x = np.asarray(x, np.float32)
    f8 = lambda v: np.asarray(v, np.float64)

    def _blk(wf8, nblk):
        # [Ein, C] -> [P, C/128 blocks, Ein/128 tiles, 128] block-major pack
        ein, c = wf8.shape
        return np.ascontiguousarray(
            wf8.reshape(ein // P, P, nblk, c // nblk).transpose(1, 2, 0, 3))

    # qkv weights: fold ln1_w, transpose to [e, col], reorder cols K|Q|V
    qkv_wp = f8(qkv_w) * f8(ln1_w)[None, :]
    wT = qkv_wp.T                                   # [E, 3E], cols Q|K|V
    wT_r = np.concatenate([wT[:, E:2 * E], wT[:, 0:E], wT[:, 2 * E:]], axis=1)
    wqkv_f8 = _to_f8(wT_r)                          # [E, 3E] K|Q|V
    wkqT = _blk(wqkv_f8[:, 0:2 * E], 12)            # [P, 12, ET, 128]
    wvT = _pack_e(wqkv_f8[:, 2 * E:])               # [P, ET, E]

    ow_f8 = _to_f8(f8(out_w).T)                     # [E, E]
    owT = _blk(ow_f8, ET).reshape(P, OW_LEN)        # [P, et, fblk, 128]

    ffn_w1p = f8(ffn_w1) * f8(ln2_w)[None, :]
    w1_f8 = _to_f8(ffn_w1p.T)                       # [E, MLP]
    w1T = _blk(w1_f8, MT).reshape(P, W1_LEN)        # [P, mt, et, 128]

    w2_f8 = _to_f8(f8(ffn_w2).T)                    # [MLP, E]
    w2T = _blk(w2_f8, ET).reshape(P, W2_LEN)        # [P, et, mt, 128]

    wtail = np.concatenate([owT, w1T, w2T], axis=1)
    assert wtail.shape == (P, WTAIL)

    # biases (LN beta folded): K 0:6 | Q 6:12 | out 12:18 | b1 18:42 | b2
    qkv_b_eff = (f8(qkv_b) + f8(qkv_w) @ f8(ln1_b))
    out_b_eff = (f8(out_b) + f8(out_w) @ f8(qkv_b)[2 * E:])
    b1_eff = (f8(ffn_b1) + f8(ffn_w1) @ f8(ln2_b))
    cf32 = np.zeros((P, 182), np.float32)
    cf32[:, 0:6] = qkv_b_eff[E:2 * E].reshape(6, P).T
    cf32[:, 6:12] = qkv_b_eff[0:E].reshape(6, P).T
    cf32[:, 12:18] = out_b_eff.reshape(6, P).T
    cf32[:, 18:42] = b1_eff.reshape(MT, P).T
    cf32[:, 42:48] = np.asarray(ffn_b2, np.float32).reshape(6, P).T
    ind2 = np.zeros((65, P), np.float32)
    ind2[0, 0:D] = 1.0
    ind2[64, D:P] = 1.0
    cf32[0:65, 54:182] = ind2

    cidx = np.arange(P)[:, None]   # key (folded, within block)
    ridx = np.arange(P)[None, :]   # query (folded, within block)
    m_prev = np.where(cidx >= ridx, 0.0, MASKNEG).astype(BF16NP)
    m_diag = np.where(cidx <= ridx, 0.0, MASKNEG).astype(BF16NP)
    m_none = np.full((P, P), MASKNEG, BF16NP)

    in_maps = []
    for c in range(N_CORES):
        b, ch = divmod(c, 4)
        lo = OWN * ch - HALO
        if ch == 0:
            slab = np.concatenate(
                [np.zeros((HALO, E), np.float32), x[b, 0:OWN]], axis=0)
        else:
            slab = x[b, lo:lo + SLAB]
        xTc = np.ascontiguousarray(
            slab.T.reshape(ET, P, SLAB).transpose(1, 0, 2)).astype(BF16NP)

        # masks [key, qb, hrep, kb, q] additive
        mask = np.stack([
            np.stack([m_none if ch == 0 else m_prev, m_diag]),  # qb = 0
            np.stack([m_prev, m_diag]),                         # qb = 1
        ]).astype(BF16NP)          # [qb, kb, key, q]
        maskc = mask.transpose(2, 0, 1, 3)          # [key, qb, kb, q]
        maskc = np.broadcast_to(maskc[:, :, None], (P, 2, 2, 2, P))
        ind2b = np.zeros((P, P), BF16NP)
        ind2b[0, 0:D] = 1.0
        ind2b[64, D:P] = 1.0
        cbf16 = np.concatenate(
            [np.ascontiguousarray(maskc).reshape(P, 1024),
             np.eye(P, dtype=BF16NP), ind2b], axis=1).astype(BF16NP)

        in_maps.append({
            "xT": xTc, "wkqT": wkqT, "wvT": wvT, "wtailT": wtail,
            "cf32T": cf32, "cbf16T": cbf16,
        })
    return in_maps


def gather_output(results):
    y = np.empty((B, L, E), np.float32)
    for c in range(N_CORES):
        b, ch = divmod(c, 4)
        yc = results[c]["yT"]          # [P, ET, OWN]
        y[b, OWN * ch:OWN * (ch + 1)] = (
            yc.transpose(2, 1, 0).reshape(OWN, E))
    return y


_NC_CACHE = None


def _get_program():
    global _NC_CACHE
    if _NC_CACHE is None:
        _NC_CACHE = build_program()
    return _NC_CACHE


def kernel(**inputs):
    nc = _get_program()
    in_maps = prep_inputs(**inputs)
    res = run_bass_kernel_spmd(nc, in_maps, core_ids=list(range(N_CORES)))
    return gather_output(res.results)



# revision 80
# speedup vs baseline: 1.2178x; 1.0032x over previous
"""Trainium2 Bass kernel: transformer block with dilated (parity-strided,
banded, causal) attention, data-parallel over 8 cores (512 own tokens +
256 halo tokens per core).

v4 design (from v2 @ ~204us):
  * All GEMMs except scores/PV run fp8 e4m3 with perf_mode=DoubleRow
    (weights x64 on host, stationary operands packed block-major so every
    lhsT slice is per-partition contiguous).
  * LayerNorm is folded into the activation quantization itself:
    x_f8 = (x - mu) * rstd quantized on DVE (two tensor ops per tile pair)
    after a PE K=1 broadcast of the mu/rstd rows through the otherwise-idle
    ps_st PSUM ring.  GEMM epilogues collapse to a single ACT op
    (f(ps/64 + bias)) -- no DVE stt, no rank-1 chain tails, no negs terms.
  * LN stats are M=1 bf16 chains over x / x^2 (ACT Square), emitted
    chain-at-a-time in the DMA-wait shadow at kernel start.
  * All weights live in SBUF up front: K/Q block-major in 2 DMAs, V
    et-major in 1, out/ffn1/ffn2 in 2 gated DMAs (5.3MB) whose transfer is
    held back by a WAR hazard on a blocker tile consumed by the x_f8
    pipeline -- so they cannot contend with the critical x/qkv HBM loads
    (ungated they monopolized HBM for 15us, stalled the PE, and the HAM
    clock dropped to half speed for the whole kernel).
  * Attention (scores + additive -49152 mask matmul + exp + pipelined PV)
    is kept from v2; den extraction moved ACT->DVE; out-proj is fp8 DR
    with a partial chain overlapping late attention.
  * ACT table sets staged as in v2 (sqrt -> exp -> sqrt -> gelu) with
    dummy activations so the ~1.3us table loads stay off the critical path.
"""

import numpy as np
import ml_dtypes

import concourse.bass as bass
import concourse.bacc as bacc
import concourse.mybir as mybir
import concourse.tile as tile
from concourse.bass_utils import run_bass_kernel_spmd

BF16NP = ml_dtypes.bfloat16
F8NP = ml_dtypes.float8_e4m3
F32 = mybir.dt.float32
BF16 = mybir.dt.bfloat16
F8 = mybir.dt.float8e4
AF = mybir.ActivationFunctionType
OP = mybir.AluOpType
DR = mybir.MatmulPerfMode.DoubleRow

P = 128
B, L, E = 2, 2048, 768
ET = E // P            # 6 tiles over E
H, D = 12, 64
MLP = 4 * E            # 3072
MT = MLP // P          # 24
OWN = 512              # tokens owned per core
HALO = 256             # preceding-context tokens
SLAB = OWN + HALO      # 768
EPS = 1e-5
N_CORES = 8
MASKNEG = -49152.0     # exact in bf16; exp((s+MASKNEG)/8) == 0
CHUNKS1 = [(0, 512), (512, 256)]   # SLAB token chunks (PSUM bank = 512 f32)
CHUNKS2 = [(0, 512)]               # OWN token chunk

# wtail layout (bytes per partition into one [P, 41472] f8 tensor)
OW_OFF, OW_LEN = 0, ET * E              # 4608
W1_OFF, W1_LEN = 4608, ET * MLP         # 18432
W2_OFF, W2_LEN = 23040, MT * E          # 18432
WTAIL = 41472


def _fold2(apv):
    """[.., T] -> [.., 2, T//2] parity view of a stride-1 token axis."""
    return apv.rearrange("... (t two) -> ... two t", two=2)


def build_program():
    nc = bacc.Bacc("TRN2", target_bir_lowering=False, debug=False)

    xT = nc.dram_tensor("xT", [P, ET, SLAB], BF16, kind="ExternalInput").ap()
    wkqT = nc.dram_tensor("wkqT", [P, 12, ET, P], F8,
                          kind="ExternalInput").ap()
    wvT = nc.dram_tensor("wvT", [P, ET, E], F8, kind="ExternalInput").ap()
    wtailT = nc.dram_tensor("wtailT", [P, WTAIL], F8,
                            kind="ExternalInput").ap()
    cf32T = nc.dram_tensor("cf32T", [P, 182], F32, kind="ExternalInput").ap()
    cbf16T = nc.dram_tensor("cbf16T", [P, 1280], BF16,
                            kind="ExternalInput").ap()
    yT = nc.dram_tensor("yT", [P, ET, OWN], F32, kind="ExternalOutput").ap()

    with tile.TileContext(nc) as tc:
        _emit(tc, xT, wkqT, wvT, wtailT, cf32T, cbf16T, yT)
    nc.compile()
    return nc


def _emit(tc, xT, wkqT, wvT, wtailT, cf32T, cbf16T, yT):
    from contextlib import ExitStack
    from collections import deque
    ctx = ExitStack()
    nc = tc.nc

    sing = ctx.enter_context(tc.tile_pool(name="sing", bufs=1))
    ex_pool = ctx.enter_context(tc.tile_pool(name="ex", bufs=3))
    row_pool = ctx.enter_context(tc.tile_pool(name="rows", bufs=2))
    ft_pool = ctx.enter_context(tc.tile_pool(name="ftmp", bufs=3))
    den_pool = ctx.enter_context(tc.tile_pool(name="den", bufs=2))

    # PSUM: 8 banks total = g:3 + st:1 + sc:2 + pv:2
    ps_main = ctx.enter_context(tc.tile_pool(name="psg", bufs=3, space="PSUM"))
    ps_st = ctx.enter_context(tc.tile_pool(name="psst", bufs=1, space="PSUM"))
    ps_attn = ctx.enter_context(tc.tile_pool(name="pssc", bufs=2, space="PSUM"))
    ps_pv = ctx.enter_context(tc.tile_pool(name="pspv", bufs=2, space="PSUM"))

    # ---------------- phase 0: input DMAs + constants ----------------
    x_sb = sing.tile([P, ET, SLAB], BF16, tag="x_sb")
    for pr in range(3):
        nc.sync.dma_start(out=x_sb[:, 2 * pr:2 * pr + 2, :],
                          in_=xT[:, 2 * pr:2 * pr + 2, :])

    cf32_sb = sing.tile([P, 182], F32, tag="cf32")
    nc.sync.dma_start(out=cf32_sb, in_=cf32T)

    # K/Q weights block-major ([P, block, et, 128]) so every lhsT slice is
    # per-partition contiguous; V weights et-major (moving operand)
    wkq_sb = sing.tile([P, 12, ET, P], F8, tag="wkq")
    nc.sync.dma_start(out=wkq_sb[:, 0:6], in_=wkqT[:, 0:6])
    nc.sync.dma_start(out=wkq_sb[:, 6:12], in_=wkqT[:, 6:12])
    wv_sb = sing.tile([P, ET, E], F8, tag="wv")
    nc.sync.dma_start(out=wv_sb, in_=wvT)

    cbf16_sb = sing.tile([P, 1280], BF16, tag="cbf16")
    nc.sync.dma_start(out=cbf16_sb, in_=cbf16T)
    masks_sb = cbf16_sb[:, 0:1024].rearrange(
        "p (qb hr kb q) -> p qb hr kb q", qb=2, hr=2, kb=2)
    ident_sb = cbf16_sb[:, 1024:1152]
    ind2_sb = cbf16_sb[0:65, 1152:1280]
    # bias columns in cf32: K 0:6 | Q 6:12 | out 12:18 | b1 18:42 | b2 42:48

    ones_row = sing.tile([1, P], BF16, tag="ones_row")
    nc.vector.memset(ones_row, 1.0)
    ones_pf = sing.tile([P, 1], BF16, tag="ones_pf")
    nc.vector.memset(ones_pf, 1.0)
    eps_sb = sing.tile([1, 1], F32, tag="eps")
    nc.vector.memset(eps_sb, EPS)
    scr_sb = sing.tile([1, 4], F32, tag="scr")

    # preload the sqrt table set while input DMAs stream
    nc.scalar.activation(scr_sb[:, 0:1], eps_sb, AF.Sqrt)

    # dummy matmuls HAM-warm the PE clock while the input DMAs stream
    warm_src = sing.tile([P, 512], BF16, tag="warm_src")
    nc.gpsimd.memset(warm_src, 0.0)
    const_bf = nc.const_aps.aps[(mybir.dt.bfloat16, 1.0)]
    wps = ps_main.tile([P, 512], F32, tag="g", name="warm_ps")
    for wi in range(10):
        nc.tensor.matmul(wps[0:1, 0:512], const_bf, warm_src,
                         start=True, stop=True)

    def emit_warm_burst(n, name):
        # dependency-free matmuls straight into the PE queue: they execute
        # exactly when the FIFO reaches them, bridging dependency-wait gaps
        # so the HAM clock gate stays at 8/8 (uses the sc ring, which is
        # idle at both insertion points)
        wb = ps_attn.tile([P, 2, 2, P], F32, tag="sc", name=name)
        wv = wb.rearrange("p a b c -> p (a b c)")
        for wi in range(n):
            nc.tensor.matmul(wv[0:1, 0:512], const_bf, warm_src,
                             start=True, stop=True)

    # weight-tail prefetch, gated so its 5.3MB transfer cannot contend with
    # the critical x/qkv HBM loads: the xsq scratch shares the wtail's
    # 1-buf pool slot, so the WAR hazard holds the wtail DMAs until the
    # LN1 sq-chains have consumed x^2 (~15us in, exactly when HBM frees up)
    wt_pool = ctx.enter_context(tc.tile_pool(name="wt", bufs=1))
    xsq_host = wt_pool.tile([P, WTAIL // 2], BF16, tag="wt", name="xsq")
    xsq_sb = xsq_host[:, 0:ET * SLAB].rearrange("p (e t) -> p e t", e=ET)

    def emit_wtail_dma():
        wtail_sb = wt_pool.tile([P, WTAIL], F8, tag="wt", name="wtail")
        nc.sync.dma_start(out=wtail_sb[:, 0:W2_OFF], in_=wtailT[:, 0:W2_OFF])
        nc.sync.dma_start(out=wtail_sb[:, W2_OFF:], in_=wtailT[:, W2_OFF:])
        ow_v = wtail_sb[:, OW_OFF:OW_OFF + OW_LEN].rearrange(
            "p (e f c) -> p e f c", e=ET, f=ET)
        w1_v = wtail_sb[:, W1_OFF:W1_OFF + W1_LEN].rearrange(
            "p (m e c) -> p m e c", m=MT, e=ET)
        w2_v = wtail_sb[:, W2_OFF:W2_OFF + W2_LEN].rearrange(
            "p (e m c) -> p e m c", e=ET, m=MT)
        return ow_v, w1_v, w2_v

    # ---------------- phase 1: LN1 stats (bf16 M=1 chains) ----------------
    # squares on DVE (idle at t=0) so ACT reaches the mu copies -- and the
    # PE the mu broadcast -- the moment the stats x-chains land
    for pr in range(3):
        pa = slice(2 * pr, 2 * pr + 2)
        nc.vector.tensor_mul(xsq_sb[:, pa, :], x_sb[:, pa, :],
                             x_sb[:, pa, :])

    def emit_ln_stats(src, srcsq, chunks, name):
        st = ps_st.tile([P, 512], F32, tag="st", name=name)
        for data, roff in ((src, 0), (srcsq, 32)):
            for ci, (c0, cl) in enumerate(chunks):
                r0 = 64 * ci + roff
                for et in range(ET):
                    nc.tensor.matmul(
                        st[r0:r0 + 1, :cl], ones_pf,
                        data[:, et, c0:c0 + cl],
                        start=(et == 0), stop=(et == ET - 1),
                        tile_position=(0, r0))
        return st

    def emit_ln_mu(st, chunks, ninv, mu_row):
        for ci, (c0, cl) in enumerate(chunks):
            r0 = 64 * ci
            nc.scalar.activation(mu_row[:, c0:c0 + cl], st[r0:r0 + 1, :cl],
                                 AF.Copy, scale=ninv)

    def emit_ln_rstd(st, chunks, ninv, a_row):
        for ci, (c0, cl) in enumerate(chunks):
            r0 = 64 * ci
            musq = row_pool.tile([1, 512], F32, tag="row")
            nc.scalar.activation(musq[:, :cl], st[r0:r0 + 1, :cl], AF.Square,
                                 scale=ninv)
            var = row_pool.tile([1, 512], F32, tag="row")
            nc.vector.scalar_tensor_tensor(
                out=var[:, :cl], in0=st[r0 + 32:r0 + 33, :cl], scalar=ninv,
                in1=musq[:, :cl], op0=OP.mult, op1=OP.subtract)
            sd = row_pool.tile([1, 512], F32, tag="row")
            nc.scalar.activation(sd[:, :cl], var[:, :cl], AF.Sqrt,
                                 bias=eps_sb)
            af = row_pool.tile([1, 512], F32, tag="row")
            nc.vector.reciprocal_approx_fast(out=af[:, :cl], in_=sd[:, :cl])
            nc.vector.tensor_copy(out=a_row[:, c0:c0 + cl], in_=af[:, :cl])

    def emit_bc(rows_reps, chunks, name):
        # row -> all partitions via PE K=1 matmuls riding the idle sc ring
        # (the st ring would force the whole LN epilogue to finish first);
        # the PSUM->SBUF copies go on ACT so the DVE can start the
        # dependent normalize-quantize ops with no queued work ahead
        for ci, (c0, cl) in enumerate(chunks):
            for ri, (a_row, a_rep) in enumerate(rows_reps):
                bc = ps_attn.tile([P, 2, 2, P], F32, tag="sc",
                                  name=f"{name}{ci}{ri}")
                bcv = bc.rearrange("p a b c -> p (a b c)")
                nc.tensor.matmul(bcv[:, :cl], ones_row, a_row[:, c0:c0 + cl],
                                 start=True, stop=True)
                nc.scalar.copy(out=a_rep[:, c0:c0 + cl], in_=bcv[:, :cl])

    mu1 = sing.tile([1, SLAB], BF16, tag="mu1")
    a1row = sing.tile([1, SLAB], BF16, tag="a1row")
    mu1rep = sing.tile([P, SLAB], BF16, tag="mu1rep")
    a1rep = sing.tile([P, SLAB], BF16, tag="a1rep")
    st1 = emit_ln_stats(x_sb, xsq_sb, CHUNKS1, "st1")
    # mu broadcast fires as soon as the mu ACT lands; the centering subs
    # then overlap the rstd (sqrt/recip) path and the rstd broadcast
    emit_ln_mu(st1, CHUNKS1, 1.0 / E, mu1)
    emit_bc([(mu1, mu1rep)], CHUNKS1, "bc1m")
    # centering subs overlap the rstd (sqrt/recip) path on ACT; the sub/mul
    # emission interleave keeps the 4-buf xc ring deadlock-free (a sub may
    # only wait on a mul that is ahead of it in the DVE queue)
    x_f8 = sing.tile([P, ET, SLAB], F8, tag="x_f8")
    xcs = []
    for et in range(3):
        xc = ft_pool.tile([P, SLAB], BF16, tag="xc", bufs=4)
        nc.vector.tensor_sub(xc, x_sb[:, et, :], mu1rep)
        xcs.append(xc)
    emit_ln_rstd(st1, CHUNKS1, 1.0 / E, a1row)
    # prefetch the exp table set once LN1's rsqrt is done (before attention)
    nc.scalar.activation(scr_sb[:, 1:2], a1row[:, 0:1], AF.Exp)
    emit_warm_burst(6, "warmP")
    emit_bc([(a1row, a1rep)], CHUNKS1, "bc1a")
    for et in range(3, ET):
        nc.vector.tensor_mul(x_f8[:, et - 3, :], xcs[et - 3], a1rep)
        xc = ft_pool.tile([P, SLAB], BF16, tag="xc", bufs=4)
        nc.vector.tensor_sub(xc, x_sb[:, et, :], mu1rep)
        xcs.append(xc)
    for et in range(3, ET):
        nc.vector.tensor_mul(x_f8[:, et, :], xcs[et], a1rep)
    ow_v, w1_v, w2_v = emit_wtail_dma()
    emit_warm_burst(10, "warmK")

    # ---------------- phase 2: QKV projections (fp8 DR, LN pre-folded) ----
    k_sb = sing.tile([P, ET, SLAB], BF16, tag="k_sb")
    q_sb = sing.tile([P, ET, OWN], BF16, tag="q_sb")
    v_sb = sing.tile([P, 2, 3, H, D + 1], BF16, tag="v_sb")
    nc.vector.memset(v_sb[:, :, :, :, D:D + 1], 1.0)
    o_sb = sing.tile([P, ET, OWN], BF16, tag="o_sb")
    o_f8 = sing.tile([P, ET, OWN], F8, tag="o_f8")
    r_all = sing.tile([65, ET, 2, 256], BF16, tag="r_all")
    # rows 1-63 are never written but ARE read by the fused K=65 rrep
    # broadcast below (against ind2's zero rows); zero them once so
    # uninitialized SBUF can't contribute 0*NaN
    nc.gpsimd.memset(r_all, 0.0)
    raw_den = [sing.tile([1, ET, 2, 256], F32, tag=f"rawden{s}",
                         name=f"raw_den{s}") for s in range(2)]
    y1_sb = sing.tile([P, ET, OWN], F32, tag="y1_sb")
    y1b_sb = sing.tile([P, ET, OWN], BF16, tag="y1b")
    y1sq_sb = sing.tile([P, ET, OWN], BF16, tag="y1sq")
    y1_f8 = sing.tile([P, ET, OWN], F8, tag="y1_f8")
    ffnh = sing.tile([P, MT, OWN], F8, tag="ffnh")

    k_ps = {}

    def emit_k(ft, defer_epi=False):
        for c0, cl in CHUNKS1:
            ps = ps_main.tile([P, 512], F32, tag="g", name=f"kps{ft}_{c0}")
            for e2 in range(0, ET, 2):
                nc.tensor.matmul(ps[:, :cl], wkq_sb[:, ft, e2:e2 + 2, :],
                                 x_f8[:, e2:e2 + 2, c0:c0 + cl],
                                 start=(e2 == 0), stop=(e2 == ET - 2),
                                 perf_mode=DR)
            if defer_epi:
                k_ps[(ft, c0)] = ps
            else:
                emit_k_epi_one(ft, c0, cl, ps)

    def emit_k_epi_one(ft, c0, cl, ps):
        # epilogue on DVE (not ACT) so attention-phase exps never queue
        nc.vector.tensor_scalar(
            out=k_sb[:, ft, c0:c0 + cl], in0=ps[:, :cl],
            scalar1=1.0 / 64.0, scalar2=cf32_sb[:, ft:ft + 1],
            op0=OP.mult, op1=OP.add)

    def emit_k_epi(ft):
        # deferred past the attention group tail so the den reciprocals
        # reach the front of the in-order DVE queue before these
        for c0, cl in CHUNKS1:
            emit_k_epi_one(ft, c0, cl, k_ps.pop((ft, c0)))

    def emit_q(ft):
        ps = ps_main.tile([P, 512], F32, tag="g", name=f"qps{ft}")
        for e2 in range(0, ET, 2):
            nc.tensor.matmul(ps, wkq_sb[:, 6 + ft, e2:e2 + 2, :],
                             x_f8[:, e2:e2 + 2, HALO:SLAB],
                             start=(e2 == 0), stop=(e2 == ET - 2),
                             perf_mode=DR)
        nc.vector.tensor_scalar(
            out=q_sb[:, ft, :], in0=ps, scalar1=1.0 / 64.0,
            scalar2=cf32_sb[:, 6 + ft:7 + ft], op0=OP.mult, op1=OP.add)

    def emit_v(ci):
        # V in [token, feature] orientation; 1/64 weight descale via ACT
        vc0, vcl = [(0, 512), (512, 256)][ci]
        for kb in range(3):
            for par in range(2):
                ps = ps_main.tile([P, 512], F32, tag="g", name=f"vps{ci}")
                for e2 in range(0, ET, 2):
                    xblk = _fold2(x_f8[:, e2:e2 + 2, :])[:, :, par,
                                                         kb * P:(kb + 1) * P]
                    nc.tensor.matmul(
                        ps[:, :vcl], xblk,
                        wv_sb[:, e2:e2 + 2, vc0:vc0 + vcl],
                        start=(e2 == 0), stop=(e2 == ET - 2), perf_mode=DR)
                # V epilogue on DVE: the ACT queue must stay clear for the
                # attention exps (a queued V copy delays PSUM frees and
                # stalls the next chain's ring slot)
                nc.vector.tensor_scalar_mul(
                    v_sb[:, par, kb, vc0 // D:(vc0 + vcl) // D, 0:D],
                    ps[:, :vcl].rearrange("p (h d) -> p h d", d=D),
                    1.0 / 64.0)

    pairs = [(0, 2), (1, 3), (4, 6), (5, 7), (8, 10), (9, 11)]

    def emit_scores(pi, par, qb):
        h0, h1 = pairs[pi]
        ro = D * (h0 % 2)
        sc = ps_attn.tile([P, 2, 2, P], F32, tag="sc", name=f"sc{pi}{par}{qb}")
        nc.tensor.matmul(sc, ident_sb, masks_sb[:, qb],
                         start=True, stop=False)
        for hi, h in enumerate((h0, h1)):
            ktt = h // 2
            qv = _fold2(q_sb[ro:ro + D, ktt, :])[:, par, qb * P:(qb + 1) * P]
            kv = _fold2(k_sb[ro:ro + D, ktt, :])
            for kbi, kb in enumerate((qb, qb + 1)):
                nc.tensor.matmul(sc[:, hi, kbi, :],
                                 kv[:, par, kb * P:(kb + 1) * P], qv,
                                 start=False, stop=(hi == 1 and kbi == 1))
        ex = ex_pool.tile([P, 2, 2, P], BF16, tag="ex", name=f"ex{pi}{par}{qb}")
        nc.scalar.activation(ex, sc, AF.Exp, scale=1.0 / np.sqrt(D))
        return ex

    def emit_pv(pi, par, qb, ex):
        h0, h1 = pairs[pi]
        kt = h0 // 2
        ro = D * (h0 % 2)
        slot = h0 % 2
        pv = ps_pv.tile([D + 1, 2, P], F32, tag="pv")
        for hi, h in enumerate((h0, h1)):
            for kbi, kb in enumerate((qb, qb + 1)):
                nc.tensor.matmul(pv[:, hi, :], v_sb[:, par, kb, h, :],
                                 ex[:, hi, kbi, :],
                                 start=(hi == 0 and kbi == 0),
                                 stop=(hi == 1 and kbi == 1))
        nc.vector.tensor_copy(
            out=raw_den[slot][0:1, kt:kt + 2, par, qb * P:(qb + 1) * P],
            in_=pv[D:D + 1, :, :])
        dst = _fold2(o_sb[ro:ro + D, kt:kt + 2, :])[:, :, par,
                                                    qb * P:(qb + 1) * P]
        nc.scalar.copy(out=dst, in_=pv[0:D])

    def emit_attn_blocks(g):
        # software-pipelined: each block's PV trails two blocks behind its
        # scores so the strict-FIFO PE queue never stalls on an in-flight exp
        blocks = [(2 * g + s, par, qb) for s in (0, 1)
                  for par in range(2) for qb in range(2)]
        pending = deque()
        ex = None
        for blk in blocks:
            ex = emit_scores(*blk)
            pending.append((blk, ex))
            if len(pending) > 2:
                b, e = pending.popleft()
                emit_pv(*b, e)
        while pending:
            b, e = pending.popleft()
            emit_pv(*b, e)
        if g == 2:
            # re-prefetch the sqrt set for LN2 after the final exp
            nc.scalar.activation(scr_sb[:, 2:3], ex[0:1, 0, 0, 0:1], AF.Sqrt)

    def emit_attn_tail(g):
        # emitted AFTER the next group's K chains: the recips/rreps wait on
        # the DVE den stream, and the interposed chains keep the PE fed
        pi = 2 * g + 1
        kt = pairs[pi][0] // 2
        # batched denominator reciprocals: one recip per head-slot over the
        # whole group's [1, 1024] raw-den slice instead of 8 per-block ones
        for s in range(2):
            rc = den_pool.tile([1, 2, 2, 256], F32, tag="rcp", bufs=1)
            nc.vector.reciprocal_approx_fast(
                out=rc.rearrange("o a b c -> o (a b c)"),
                in_=raw_den[s][0:1, kt:kt + 2, :, :]
                .rearrange("o a b c -> o (a b c)"))
            nc.vector.tensor_copy(
                out=r_all[64 * s:64 * s + 1, kt:kt + 2, :, :], in_=rc)
        for tt in (kt, kt + 1):
            rrep = ps_main.tile([P, 512], F32, tag="g", name="rrep_ps")
            rrv = rrep.rearrange("m (a q) -> m a q", a=2)
            # single K=65 matmul: ind2 row 0 routes the slot-0 denominators
            # to partitions 0-63, row 64 routes slot-1 to 64-127; the zero
            # rows in between null out the unwritten r_all lanes
            nc.tensor.matmul(rrv, ind2_sb, r_all[0:65, tt],
                             start=True, stop=True)
            ofv = _fold2(o_sb[:, tt, :])
            of8v = _fold2(o_f8[:, tt, :])
            nc.vector.tensor_mul(of8v, ofv, rrv)
        if pi == 3:
            # out-proj partial A over o feature tiles 0-3 (ready now), fp8 DR
            for et in range(ET):
                ps = ps_main.tile([P, 512], F32, tag="g", name="opA")
                for f2 in range(0, 4, 2):
                    nc.tensor.matmul(ps, ow_v[:, et, f2:f2 + 2, :],
                                     o_f8[:, f2:f2 + 2, :],
                                     start=(f2 == 0), stop=(f2 == 2),
                                     perf_mode=DR)
                t = ft_pool.tile([P, 512], F32, tag="ft")
                nc.scalar.activation(t, ps, AF.Identity, scale=1.0 / 64.0,
                                     bias=cf32_sb[:, 12 + et:13 + et])
                # residual add on gpsimd (idle during attention); frees the
                # DVE for the den/recip stream so opA's PSUM ring never gates
                nc.gpsimd.tensor_add(y1_sb[:, et, :], t,
                                     x_sb[:, et, HALO:SLAB])

    emit_k(0)
    emit_k(1)
    emit_warm_burst(4, "warmK2")
    emit_q(0)
    emit_q(1)
    emit_v(0)
    emit_warm_burst(8, "warmA")
    emit_attn_blocks(0)
    emit_k(2, defer_epi=True); emit_k(3, defer_epi=True)
    emit_attn_tail(0)
    emit_k_epi(2); emit_k_epi(3)
    emit_q(2); emit_q(3)
    emit_attn_blocks(1)
    emit_k(4, defer_epi=True); emit_k(5, defer_epi=True)
    emit_attn_tail(1)
    emit_k_epi(4); emit_k_epi(5)
    emit_q(4); emit_q(5)
    emit_v(1)
    emit_attn_blocks(2)
    emit_attn_tail(2)

    # ------- phase 4: out-proj partial B + residual + inline LN2 stats ----
    # the stat-chain matmuls ride along per-et so the PE has real work
    # during the DVE/ACT-heavy opB epilogues and mu2 lands ~4us earlier
    st2 = ps_st.tile([P, 512], F32, tag="st", name="st2")
    for et in range(ET):
        ps = ps_main.tile([P, 512], F32, tag="g")
        nc.tensor.matmul(ps, ow_v[:, et, 4:6, :],
                         o_f8[:, 4:6, :], start=True, stop=True,
                         perf_mode=DR)
        nc.vector.scalar_tensor_tensor(
            out=y1_sb[:, et, :], in0=ps, scalar=1.0 / 64.0,
            in1=y1_sb[:, et, :], op0=OP.mult, op1=OP.add)
        nc.scalar.copy(out=y1b_sb[:, et, :], in_=y1_sb[:, et, :])
        nc.scalar.activation(y1sq_sb[:, et, :], y1b_sb[:, et, :], AF.Square)
        nc.tensor.matmul(st2[0:1, :], ones_pf, y1b_sb[:, et, :],
                         start=(et == 0), stop=(et == ET - 1),
                         tile_position=(0, 0))
        nc.tensor.matmul(st2[32:33, :], ones_pf, y1sq_sb[:, et, :],
                         start=(et == 0), stop=(et == ET - 1),
                         tile_position=(0, 32))
        if et in (1, 4):
            emit_warm_burst(3, "warmO")

    # ---------------- phase 5: LN2 epilogue ----------------
    mu2 = sing.tile([1, OWN], BF16, tag="mu2")
    a2row = sing.tile([1, OWN], BF16, tag="a2row")
    mu2rep = sing.tile([P, OWN], BF16, tag="mu2rep")
    a2rep = sing.tile([P, OWN], BF16, tag="a2rep")
    emit_ln_mu(st2, CHUNKS2, 1.0 / E, mu2)
    emit_bc([(mu2, mu2rep)], CHUNKS2, "bc2m")
    ycs = []
    for et in range(3):
        yc = ft_pool.tile([P, SLAB], BF16, tag="xc", bufs=4)
        nc.vector.tensor_sub(yc[:, :OWN], y1b_sb[:, et, :], mu2rep)
        ycs.append(yc)
    emit_ln_rstd(st2, CHUNKS2, 1.0 / E, a2row)
    # prefetch the gelu table set now that the last exp (rstd2) is emitted
    nc.scalar.activation(scr_sb[:, 3:4], a2row[:, 0:1], AF.Gelu)
    emit_warm_burst(4, "warmB")
    emit_bc([(a2row, a2rep)], CHUNKS2, "bc2a")
    for et in range(3, ET):
        nc.vector.tensor_mul(y1_f8[:, et - 3, :], ycs[et - 3][:, :OWN], a2rep)
        yc = ft_pool.tile([P, SLAB], BF16, tag="xc", bufs=4)
        nc.vector.tensor_sub(yc[:, :OWN], y1b_sb[:, et, :], mu2rep)
        ycs.append(yc)
    for et in range(3, ET):
        nc.vector.tensor_mul(y1_f8[:, et, :], ycs[et][:, :OWN], a2rep)
        if et == 4:
            emit_warm_burst(3, "warmY")

    # ---------------- phase 6: FFN1 + GELU (fp8 DR, LN pre-folded) --------
    for mt in range(MT):
        ps = ps_main.tile([P, 512], F32, tag="g", name=f"f1_{mt}")
        for e2 in range(0, ET, 2):
            nc.tensor.matmul(ps, w1_v[:, mt, e2:e2 + 2, :],
                             y1_f8[:, e2:e2 + 2, :],
                             start=(e2 == 0), stop=(e2 == ET - 2),
                             perf_mode=DR)
        nc.scalar.activation(ffnh[:, mt, :], ps, AF.Gelu, scale=1.0 / 64.0,
                             bias=cf32_sb[:, 18 + mt:19 + mt])

    # ---------------- phase 7: FFN2 + residual + store ----------------
    for et in range(ET):
        ps = ps_main.tile([P, 512], F32, tag="g")
        for k2 in range(0, MT, 2):
            nc.tensor.matmul(ps, w2_v[:, et, k2:k2 + 2, :],
                             ffnh[:, k2:k2 + 2, :],
                             start=(k2 == 0), stop=(k2 == MT - 2),
                             perf_mode=DR)
        t = ft_pool.tile([P, 512], F32, tag="ft")
        nc.scalar.activation(t, ps, AF.Identity, scale=1.0 / 64.0,
                             bias=cf32_sb[:, 42 + et:43 + et])
        nc.vector.tensor_add(y1_sb[:, et, :], t, y1_sb[:, et, :])
        nc.sync.dma_start(out=yT[:, et, :], in_=y1_sb[:, et, :])

    ctx.close()


# ======================= host side =======================

def _to_f8(w):
    return np.clip(w * 64.0, -240.0, 240.0).astype(F8NP)


def _pack_e(wT):
    """[E, C] (contraction-major) -> [P, ET, C] partition pack."""
    C = wT.shape[1]
    return np.ascontiguousarray(
        wT.reshape(ET, P, C).transpose(1, 0, 2))


def prep_inputs(x, ln1_w, ln1_b, qkv_w, qkv_b, out_w, out_b,
                ln2_w, ln2_b, ffn_w1, ffn_b1, ffn_w2, ffn_b2):
    """Shard/fold/cast the full inputs into 8 per-core input maps."""
    x = np.asarray(x, np.float32)
    f8 = lambda v: np.asarray(v, np.float64)

    def _blk(wf8, nblk):
        # [Ein, C] -> [P, C/128 blocks, Ein/128 tiles, 128] block-major pack
        ein, c = wf8.shape
        return np.ascontiguousarray(
            wf8.reshape(ein // P, P, nblk, c // nblk).transpose(1, 2, 0, 3))

    # qkv weights: fold ln1_w, transpose to [e, col], reorder cols K|Q|V
    qkv_wp = f8(qkv_w) * f8(ln1_w)[None, :]
    wT = qkv_wp.T                                   # [E, 3E], cols Q|K|V
    wT_r = np.concatenate([wT[:, E:2 * E], wT[:, 0:E], wT[:, 2 * E:]], axis=1)
    wqkv_f8 = _to_f8(wT_r)                          # [E, 3E] K|Q|V
    wkqT = _blk(wqkv_f8[:, 0:2 * E], 12)            # [P, 12, ET, 128]
    wvT = _pack_e(wqkv_f8[:, 2 * E:])               # [P, ET, E]

    ow_f8 = _to_f8(f8(out_w).T)                     # [E, E]
    owT = _blk(ow_f8, ET).reshape(P, OW_LEN)        # [P, et, fblk, 128]

    ffn_w1p = f8(ffn_w1) * f8(ln2_w)[None, :]
    w1_f8 = _to_f8(ffn_w1p.T)                       # [E, MLP]
    w1T = _blk(w1_f8, MT).reshape(P, W1_LEN)        # [P, mt, et, 128]

    w2_f8 = _to_f8(f8(ffn_w2).T)                    # [MLP, E]
    w2T = _blk(w2_f8, ET).reshape(P, W2_LEN)        # [P, et, mt, 128]

    wtail = np.concatenate([owT, w1T, w2T], axis=1)
    assert wtail.shape == (P, WTAIL)

    # biases (LN beta folded): K 0:6 | Q 6:12 | out 12:18 | b1 18:42 | b2
    qkv_b_eff = (f8(qkv_b) + f8(qkv_w) @ f8(ln1_b))
    out_b_eff = (f8(out_b) + f8(out_w) @ f8(qkv_b)[2 * E:])
    b1_eff = (f8(ffn_b1) + f8(ffn_w1) @ f8(ln2_b))
    cf32 = np.zeros((P, 182), np.float32)
    cf32[:, 0:6] = qkv_b_eff[E:2 * E].reshape(6, P).T
    cf32[:, 6:12] = qkv_b_eff[0:E].reshape(6, P).T
    cf32[:, 12:18] = out_b_eff.reshape(6, P).T
    cf32[:, 18:42] = b1_eff.reshape(MT, P).T
    cf32[:, 42:48] = np.asarray(ffn_b2, np.float32).reshape(6, P).T
    ind2 = np.zeros((65, P), np.float32)
    ind2[0, 0:D] = 1.0
    ind2[64, D:P] = 1.0
    cf32[0:65, 54:182] = ind2

    cidx = np.arange(P)[:, None]   # key (folded, within block)
    ridx = np.arange(P)[None, :]   # query (folded, within block)
    m_prev = np.where(cidx >= ridx, 0.0, MASKNEG).astype(BF16NP)
    m_diag = np.where(cidx <= ridx, 0.0, MASKNEG).astype(BF16NP)
    m_none = np.full((P, P), MASKNEG, BF16NP)

    in_maps = []
    for c in range(N_CORES):
        b, ch = divmod(c, 4)
        lo = OWN * ch - HALO
        if ch == 0:
            slab = np.concatenate(
                [np.zeros((HALO, E), np.float32), x[b, 0:OWN]], axis=0)
        else:
            slab = x[b, lo:lo + SLAB]
        xTc = np.ascontiguousarray(
            slab.T.reshape(ET, P, SLAB).transpose(1, 0, 2)).astype(BF16NP)

        # masks [key, qb, hrep, kb, q] additive
        mask = np.stack([
            np.stack([m_none if ch == 0 else m_prev, m_diag]),  # qb = 0
            np.stack([m_prev, m_diag]),                         # qb = 1
        ]).astype(BF16NP)          # [qb, kb, key, q]
        maskc = mask.transpose(2, 0, 1, 3)          # [key, qb, kb, q]
        maskc = np.broadcast_to(maskc[:, :, None], (P, 2, 2, 2, P))
        ind2b = np.zeros((P, P), BF16NP)
        ind2b[0, 0:D] = 1.0
        ind2b[64, D:P] = 1.0
        cbf16 = np.concatenate(
            [np.ascontiguousarray(maskc).reshape(P, 1024),
             np.eye(P, dtype=BF16NP), ind2b], axis=1).astype(BF16NP)

        in_maps.append({
            "xT": xTc, "wkqT": wkqT, "wvT": wvT, "wtailT": wtail,
            "cf32T": cf32, "cbf16T": cbf16,
        })
    return in_maps


def gather_output(results):
    y = np.empty((B, L, E), np.float32)
    for c in range(N_CORES):
        b, ch = divmod(c, 4)
        yc = results[c]["yT"]          # [P, ET, OWN]
        y[b, OWN * ch:OWN * (ch + 1)] = (
            yc.transpose(2, 1, 0).reshape(OWN, E))
    return y


_NC_CACHE = None


def _get_program():
    global _NC_CACHE
    if _NC_CACHE is None:
        _NC_CACHE = build_program()
    return _NC_CACHE


def kernel(**inputs):
    nc = _get_program()
    in_maps = prep_inputs(**inputs)
    res = run_bass_kernel_spmd(nc, in_maps, core_ids=list(range(N_CORES)))
    return gather_output(res.results)

